# revision 18
# baseline (speedup 1.0000x reference)
"""Trainium2 Bass kernel for an attention-augmented GRU cell (CGRUCell).

Reference computation (per batch row):
    cache   = context @ Wk.T + bk                  # [S, A]
    q       = hidden @ Wq.T + bq                   # [A]
    logits  = tanh(q + cache) @ Wl[0] + bl         # [S]
    logits  = where(mask, -1e18, logits)
    w       = softmax(logits)                      # [S]
    attn    = w @ context                          # [CTX]
    x       = input @ We.T + be + attn @ Wa.T + ba
    gx      = x @ W_ih.T + b_ih ; gh = hidden @ W_hh.T + b_hh
    r, z    = sigmoid(gx_r + gh_r), sigmoid(gx_z + gh_z)
    n       = tanh(gx_n + r * gh_n)
    hidden1 = (1 - z) * n + z * hidden
Outputs: (hidden1, attn)

Strategy: data-parallel over batch on 8 NeuronCores (8 rows each). The
dominant work is the [S,CTX]@[CTX,A] key projection; it runs on the
TensorEngine in fp8e4 DoubleRow mode (two contraction tiles per pass,
2x bf16 rate) off a host-pre-transposed fp8 copy of context, with Wk
host-scaled by 64 and the 1/64 folded into the tanh activation's
scale. The softmax reduction over the attention dim rides on bf16
matmuls against a 128-replicated Wl with the q/bk bias fused into the
tanh Activation op; the attention-value matvec contracts a bf16
natural-layout copy of context. The GRU algebra is reassociated
(W1 = W_ih@We, W2 = W_ih@Wa) so all of it except attn @ W2.T is
computed from the raw inputs by weight-stationary matmuls pipelined
through the batch rows. Each row's softmax/attention tail is emitted
interleaved into the next row's cache matmuls so the in-order PE never
idles on DVE/ACT latency. All host-staged tensors are laid out so
every DMA lands with contiguous per-partition chunks (strided-element
DMAs cost ~100x).
"""

import sys

if "/opt/trn_rl_repo" not in sys.path:
    sys.path.insert(0, "/opt/trn_rl_repo")

import ml_dtypes
import numpy as np

import concourse.bass as bass
import concourse.tile as tile
from concourse import bacc, mybir
from concourse.bass_utils import run_bass_kernel_spmd

NCORES = 8
B, S, IN, HID, CTX, ATT = 64, 1024, 1024, 1024, 1024, 1024
BL = B // NCORES          # batch rows per core
H3 = 3 * HID
AT, CT, HT, H3T = ATT // 128, CTX // 128, HID // 128, H3 // 128  # 8,8,8,24
ST = S // 128
F32 = mybir.dt.float32
BF16 = mybir.dt.bfloat16
FP8 = mybir.dt.float8e4
AX = mybir.AxisListType
AF = mybir.ActivationFunctionType
DR = mybir.MatmulPerfMode.DoubleRow
BF16NP = ml_dtypes.bfloat16
FP8NP = ml_dtypes.float8_e4m3
WK_SCALE = 64.0
AT2_SCALE = 32.0
W2_SCALE = 64.0


def build_program():
    nc = bacc.Bacc("TRN2", target_bir_lowering=False, debug=False, num_devices=NCORES)

    d_ctxb = nc.dram_tensor("ctxb", [BL, 128, ST, CTX], BF16, kind="ExternalInput").ap()
    d_ctxT8 = nc.dram_tensor("ctxT8", [BL, 128, CT, S], FP8, kind="ExternalInput").ap()
    d_pen = nc.dram_tensor("pen", [1, BL * S], BF16, kind="ExternalInput").ap()
    d_wlrep = nc.dram_tensor("wlrep", [128, AT * 128], BF16, kind="ExternalInput").ap()
    d_ones1 = nc.dram_tensor("ones1", [1, 128], BF16, kind="ExternalInput").ap()
    d_wk8 = nc.dram_tensor("wk8", [128, CT, ATT], FP8, kind="ExternalInput").ap()
    d_wgall = nc.dram_tensor(
        "wgall", [28, 128, HT * 2 * 128], BF16, kind="ExternalInput"
    ).ap()
    d_w2g = nc.dram_tensor(
        "w2g", [12, 128, CT, 2 * 128], FP8, kind="ExternalInput"
    ).ap()
    d_hT = nc.dram_tensor("hT", [128, HT * BL], BF16, kind="ExternalInput").ap()
    d_inT = nc.dram_tensor("inT", [128, HT * BL], BF16, kind="ExternalInput").ap()
    d_identf = nc.dram_tensor("identf", [128, 128], F32, kind="ExternalInput").ap()
    d_one1b = nc.dram_tensor("one1b", [1, 1], BF16, kind="ExternalInput").ap()
    d_one32b = nc.dram_tensor("one32b", [1, 1], BF16, kind="ExternalInput").ap()
    d_hTf = nc.dram_tensor("hTf", [128, HT * BL], F32, kind="ExternalInput").ap()
    d_bqk = nc.dram_tensor("bqk", [128, AT], F32, kind="ExternalInput").ap()
    d_bx = nc.dram_tensor("bx", [128, H3T], F32, kind="ExternalInput").ap()
    d_bhh = nc.dram_tensor("bhh", [128, H3T], F32, kind="ExternalInput").ap()

    d_h1 = nc.dram_tensor("h1", [BL, HID], F32, kind="ExternalOutput").ap()
    d_attn = nc.dram_tensor("attn", [BL, CTX], F32, kind="ExternalOutput").ap()

    with tile.TileContext(nc) as tc:
        _emit(tc, locals())
    nc.compile()
    return nc


def _emit(tc, d):
    from contextlib import ExitStack

    nc = tc.nc
    CP = CT // 2  # contraction-tile pairs for DoubleRow

    stack = ExitStack()
    pool = lambda *a, **k: stack.enter_context(tc.tile_pool(*a, **k))
    cst = pool(name="cst", bufs=1)
    actp = pool(name="actp", bufs=1)
    wkp = pool(name="wkp", bufs=1)
    wstream = pool(name="wstream", bufs=6)
    natp = pool(name="natp", bufs=3)
    nat8p = pool(name="nat8p", bufs=2)
    tanhp = pool(name="tanhp", bufs=6)
    expp = pool(name="expp", bufs=2)
    arowp = pool(name="arowp", bufs=3)
    ecolp = pool(name="ecolp", bufs=2)
    smallp = pool(name="smallp", bufs=6)
    w2p = pool(name="w2p", bufs=5)

    # PSUM pools: 8 banks total (pc 4 + pl 2 + shared scratch 2)
    pc_ps = pool(name="pc_ps", bufs=4, space="PSUM")
    pl_ps = pool(name="pl_ps", bufs=2, space="PSUM")
    ms_ps = pool(name="ms_ps", bufs=2, space="PSUM")

    # ---- urgent loads first, spread across the three DMA queues:
    # gpsimd: nat8(0); scalar: wk then small consts then natb(0);
    # sync: hT then the pipelined weight-group stream.
    nat8_0 = nat8p.tile([128, CT, S], FP8, tag="nat8", name="nat8")
    wk_sb = wkp.tile([128, CT, ATT], FP8, tag="wk")
    for c2 in range(CT // 2):
        sl2 = slice(2 * c2, 2 * c2 + 2)
        nc.gpsimd.dma_start(nat8_0[:, sl2, :], d["d_ctxT8"][0][:, sl2, :])
        nc.scalar.dma_start(wk_sb[:, sl2, :], d["d_wk8"][:, sl2, :])
    hT_sb = actp.tile([128, HT * BL], BF16, tag="hT")
    nc.sync.dma_start(hT_sb[:], d["d_hT"][:])

    loads = {}

    def preload(b):
        natb = natp.tile([128, ST, CTX], BF16, tag="natb", name="natb")
        nc.scalar.dma_start(natb[:], d["d_ctxb"][b])
        nat8 = nat8p.tile([128, CT, S], FP8, tag="nat8", name="nat8")
        nc.gpsimd.dma_start(nat8[:], d["d_ctxT8"][b])
        loads[b] = (natb, nat8)

    # ---- small constants on the scalar queue, most-urgent first ----
    bqk_sb = cst.tile([128, AT], F32, tag="bqk")
    nc.scalar.dma_start(bqk_sb[:], d["d_bqk"][:])
    wlrep_sb = cst.tile([128, AT * 128], BF16, tag="wlrep")
    nc.scalar.dma_start(wlrep_sb[:], d["d_wlrep"][:])
    ones1 = cst.tile([1, 128], BF16, tag="ones1")
    nc.scalar.dma_start(ones1[:], d["d_ones1"][:])
    one1b = cst.tile([1, 1], BF16, tag="one1b")
    nc.scalar.dma_start(one1b[:], d["d_one1b"][:])
    one32b = cst.tile([1, 1], BF16, tag="one32b")
    nc.scalar.dma_start(one32b[:], d["d_one32b"][:])
    pen_sb = cst.tile([1, BL * S], BF16, tag="pen")
    nc.scalar.dma_start(pen_sb[:], d["d_pen"][:])
    inT_sb = actp.tile([128, HT * BL], BF16, tag="inT")
    nc.scalar.dma_start(inT_sb[:], d["d_inT"][:])

    # natb(0) next (first needed at a==4 of row 0)
    natb_0 = natp.tile([128, ST, CTX], BF16, tag="natb", name="natb")
    nc.scalar.dma_start(natb_0[:], d["d_ctxb"][0])
    loads[0] = (natb_0, nat8_0)

    # tail-only constants: tiles declared here, DMAs deferred to row 1
    identf = cst.tile([128, 128], F32, tag="identf")
    bx_sb = cst.tile([128, H3T], F32, tag="bx")
    bhh_sb = cst.tile([128, H3T], F32, tag="bhh")
    hTf_sb = actp.tile([128, HT * BL], F32, tag="hTf")

    def load_tail_consts():
        nc.gpsimd.dma_start(identf[:], d["d_identf"][:])
        nc.gpsimd.dma_start(bx_sb[:], d["d_bx"][:])
        nc.gpsimd.dma_start(bhh_sb[:], d["d_bhh"][:])
        nc.gpsimd.dma_start(hTf_sb[:], d["d_hTf"][:])

    # ---- weight-stationary GEMMs: qeff = Wq@hT + (bq+bk); gx1 = W1@inT + bx;
    # gh = Whh@hT + bhh. DMA and matmul emission are pipelined (DMA runs two
    # groups ahead) and spread through the batch rows so the many small
    # LDWEIGHTS never starve the cache matmul stream.
    qeff = actp.tile([128, AT * BL], F32, tag="qeff")
    gx1 = actp.tile([128, H3T * BL], F32, tag="gx1")
    gh = actp.tile([128, H3T * BL], F32, tag="gh")
    TG = 2  # output tiles per weight DMA

    wspecs = [(qeff, bqk_sb, hT_sb, t0) for t0 in range(0, AT, TG)]
    wspecs += [(gx1, bx_sb, inT_sb, t0) for t0 in range(0, H3T, TG)]
    wspecs += [(gh, bhh_sb, hT_sb, t0) for t0 in range(0, H3T, TG)]
    wt_tiles = {}
    wcnt = {"dma": 0, "mm": 0}

    def dma_wgroup():
        k = wcnt["dma"]
        if k >= len(wspecs) or k - wcnt["mm"] >= 5:
            return
        wcnt["dma"] += 1
        wt = wstream.tile([128, HT * TG * 128], BF16, tag="ws", name="wt")
        eng = nc.sync if k % 2 == 0 else nc.gpsimd
        eng.dma_start(wt[:], d["d_wgall"][k])
        wt_tiles[k] = wt

    def mm_wgroup():
        k = wcnt["mm"]
        if k >= len(wspecs):
            return
        wcnt["mm"] += 1
        dst, bias_sb, rhs, t0 = wspecs[k]
        wt = wt_tiles.pop(k)
        for tl in range(TG):
            t = t0 + tl
            pg = ms_ps.tile([128, BL], F32, tag="ms")
            for j in range(HT):
                lhs = wt[:, j * TG * 128 + tl * 128 : j * TG * 128 + (tl + 1) * 128]
                nc.tensor.matmul(
                    pg[:], lhs, rhs[:, j * BL : (j + 1) * BL],
                    start=(j == 0), stop=(j == HT - 1),
                )
            nc.vector.tensor_scalar_add(
                dst[:, t * BL : (t + 1) * BL], pg[:], bias_sb[:, t : t + 1]
            )

    # ---- main attention loop over local batch rows ----
    # Batch row b's softmax/attention tail is deferred and emitted at
    # checkpoints inside row b+1's cache-matmul loop so the in-order PE
    # always has dense matmul work while DVE/ACT chase the softmax
    # dependency chain.
    sums = actp.tile([128, BL], F32, tag="sums")
    recip = actp.tile([128, BL], F32, tag="recip")
    attnT = actp.tile([128, CT, BL], FP8, tag="attnT")
    deferred = []  # closures carrying batch b-1's softmax/attn chunks
    w2tiles = []

    def make_chunks(b, natb, plb0, plb1):
        state = {}

        def run1():  # max + exp straight off the broadcast-logits psum
            mx2 = smallp.tile([128, 2], F32, tag="mx2")
            nc.vector.reduce_max(mx2[:, 0:1], plb0[:], axis=AX.X)
            nc.vector.reduce_max(mx2[:, 1:2], plb1[:], axis=AX.X)
            nmx = smallp.tile([128, 1], F32, tag="nmx")
            nc.vector.reduce_max(nmx[:], mx2[:], axis=AX.X, negate=True)
            acc2 = smallp.tile([128, 2], F32, tag="acc2")
            etile = expp.tile([128, S], BF16, tag="exp")
            nc.scalar.activation(
                etile[:, 0:512], plb0[:], AF.Exp, bias=nmx[:], accum_out=acc2[:, 0:1]
            )
            nc.scalar.activation(
                etile[:, 512:1024], plb1[:], AF.Exp, bias=nmx[:], accum_out=acc2[:, 1:2]
            )
            nc.vector.tensor_add(sums[:, b : b + 1], acc2[:, 0:1], acc2[:, 1:2])
            nc.vector.reciprocal(recip[:, b : b + 1], sums[:, b : b + 1])
            state["etile"] = etile

        def run2():  # exp row -> column layout for the matvec
            etile = state["etile"]
            pe = ms_ps.tile([128, ST], F32, tag="ms")
            for st in range(ST):
                nc.tensor.matmul(
                    pe[:, st : st + 1],
                    etile[0:1, 128 * st : 128 * (st + 1)],
                    one1b[:],
                    start=True, stop=True,
                )
            ecol = ecolp.tile([128, ST], BF16, tag="ecol")
            nc.vector.tensor_copy(ecol[:], pe[:])
            state["ecol"] = ecol

        def run3():  # attention values + normalized output row
            ecol = state["ecol"]
            arow = arowp.tile([1, CTX], F32, tag="arow")
            for cg in range(2):
                pav = ms_ps.tile([1, 512], F32, tag="ms")
                for st in range(ST):
                    nc.tensor.matmul(
                        pav[:], ecol[:, st : st + 1],
                        natb[:, st, 512 * cg : 512 * (cg + 1)],
                        start=(st == 0), stop=(st == ST - 1),
                    )
                nc.vector.tensor_copy(arow[:, 512 * cg : 512 * (cg + 1)], pav[:])
            an = arowp.tile([1, CTX], F32, tag="arow")
            nc.vector.tensor_scalar_mul(an[:], arow[:], recip[0:1, b : b + 1])
            nc.sync.dma_start(d["d_attn"][b : b + 1, :], an[:])
            arowb = arowp.tile([1, CTX], BF16, tag="arow")
            nc.vector.tensor_scalar_mul(arowb[:], an[:], AT2_SCALE)
            state["arowb"] = arowb

        def run4():  # attnT columns for the W2 matmul
            arowb = state["arowb"]
            pat = ms_ps.tile([128, CT], F32, tag="ms")
            for c in range(CT):
                nc.tensor.matmul(
                    pat[:, c : c + 1],
                    arowb[0:1, 128 * c : 128 * (c + 1)],
                    one1b[:],
                    start=True, stop=True,
                )
            nc.vector.tensor_copy(attnT[:, :, b], pat[:])

        return [run1, run2, run3, run4]

    for _ in range(4):
        dma_wgroup()
    mm_wgroup()  # qeff tiles 0-1, needed by row 0's first tanh
    finish_prev = None
    for b in range(BL):
        natb, nat8 = loads.pop(b)
        if b == 1:
            load_tail_consts()

        if b == BL - 1:
            # pre-issue the tail's W2 stream on the now-idle SWDGE queue so
            # the gxa matmuls aren't DMA-paced after the last batch row
            for t0 in range(0, H3T, TG):
                wt2 = w2p.tile([128, CT, TG * 128], FP8, tag="w2s", name="wt2")
                nc.gpsimd.dma_start(wt2[:], d["d_w2g"][t0 // TG])
                w2tiles.append(wt2)

        # cache matmul (fp8 DoubleRow) + tanh + broadcast-logits reduction.
        # The replicated Wl stationary operand makes the Wl-contraction emit
        # logits replicated across all 128 partitions, ready for softmax.
        # pl matmuls for a-1 are emitted after the cache matmuls of a so the
        # PE never waits on the tanh ACT drain.
        plb0 = pl_ps.tile([128, 512], F32, tag="pl")
        plb1 = pl_ps.tile([128, 512], F32, tag="pl")
        pending = []

        def emit_pl(th0, th1, a, plb0=plb0, plb1=plb1):
            lhs = wlrep_sb[:, a * 128 : (a + 1) * 128]
            nc.tensor.matmul(plb0[:], lhs, th0[:], start=(a == 0), stop=False)
            nc.tensor.matmul(plb1[:], lhs, th1[:], start=(a == 0), stop=False)

        for a in range(AT):
            pc0 = pc_ps.tile([128, 512], F32, tag="pc")
            pc1 = pc_ps.tile([128, 512], F32, tag="pc")
            for cp in range(CP):
                lhs = wk_sb[:, 2 * cp : 2 * cp + 2, 128 * a : 128 * (a + 1)]
                nc.tensor.matmul(
                    pc0[:], lhs, nat8[:, 2 * cp : 2 * cp + 2, 0:512],
                    start=(cp == 0), stop=(cp == CP - 1), perf_mode=DR,
                )
                nc.tensor.matmul(
                    pc1[:], lhs, nat8[:, 2 * cp : 2 * cp + 2, 512:1024],
                    start=(cp == 0), stop=(cp == CP - 1), perf_mode=DR,
                )
            if a == 0 and finish_prev is not None:
                finish_prev()  # prev batch's pl(7) + penalty matmuls
            if deferred:
                if a == 1:
                    deferred[0]()  # DVE/ACT only: max+exp off the pl psum
                elif a == 2:
                    deferred[1]()
                elif a == 4:
                    deferred[2]()
                elif a == 6:
                    deferred[3]()
            if a == 1 and b + 1 < BL:
                preload(b + 1)
            if len(pending) >= 2:
                emit_pl(*pending.pop(0))
            if a < 4:
                dma_wgroup()
                if (b >= 2 or wcnt["mm"] < 4) and wcnt["mm"] < len(wspecs) - 3:
                    mm_wgroup()
            th0 = tanhp.tile([128, 512], BF16, tag="tanh")
            th1 = tanhp.tile([128, 512], BF16, tag="tanh")
            qcol = qeff[:, a * BL + b : a * BL + b + 1]
            nc.scalar.activation(
                th0[:], pc0[:], AF.Tanh, bias=qcol, scale=1.0 / WK_SCALE
            )
            nc.scalar.activation(
                th1[:], pc1[:], AF.Tanh, bias=qcol, scale=1.0 / WK_SCALE
            )
            pending.append((th0, th1, a))

        def finish_prev(
            pending=pending, emit_pl=emit_pl, plb0=plb0, plb1=plb1, b=b
        ):
            for p in pending:
                emit_pl(*p)
            # fold the additive mask penalties into the broadcast logits
            nc.tensor.matmul(
                plb0[:], ones1[:], pen_sb[0:1, b * S : b * S + 512], start=False, stop=True
            )
            nc.tensor.matmul(
                plb1[:], ones1[:], pen_sb[0:1, b * S + 512 : b * S + 1024], start=False, stop=True
            )

        deferred = make_chunks(b, natb, plb0, plb1)

    finish_prev()  # flush last batch row
    for fn in deferred:
        fn()
        dma_wgroup()
        mm_wgroup()  # reserved weight groups keep the PE fed during softmax
    while wcnt["mm"] < len(wspecs):
        dma_wgroup()
        mm_wgroup()

    # ---- tail: gxa = W2 @ attnT (attnT already normalized, fp8 x32; W2
    # fp8 x64 -> psum carries 2048x, folded out in the Identity copy). The
    # gate math is emitted per r/z/n section as soon as that section's gxa
    # groups land, so DVE/ACT overlap the LDWEIGHTS-paced gxa stream.
    W = HT * BL  # 64
    gxa_all = actp.tile([128, H3T * BL], F32, tag="gxa_all")
    h1nat = actp.tile([BL, HID], F32, tag="h1nat")
    r_all = actp.tile([128, W], F32, tag="r_all")
    z_all = actp.tile([128, W], F32, tag="z_all")
    gxfn = actp.tile([128, W], F32, tag="gxfn")

    def gates_r():
        rz = actp.tile([128, W], F32, tag="rz")
        nc.vector.tensor_add(rz[:], gxa_all[:, 0:W], gx1[:, 0:W])
        nc.vector.tensor_add(rz[:], rz[:], gh[:, 0:W])
        nc.scalar.activation(r_all[:], rz[:], AF.Sigmoid)

    def gates_z():
        rz = actp.tile([128, W], F32, tag="rz")
        nc.vector.tensor_add(rz[:], gxa_all[:, W : 2 * W], gx1[:, W : 2 * W])
        nc.vector.tensor_add(rz[:], rz[:], gh[:, W : 2 * W])
        nc.scalar.activation(z_all[:], rz[:], AF.Sigmoid)

    section_done = {8: gates_r, 16: gates_z}
    for t0 in range(0, H3T, TG):
        wt2 = w2tiles[t0 // TG]
        for tl in range(TG):
            t = t0 + tl
            pg = ms_ps.tile([128, BL], F32, tag="ms")
            for cp in range(CT // 2):
                lhs = wt2[:, 2 * cp : 2 * cp + 2, tl * 128 : (tl + 1) * 128]
                nc.tensor.matmul(
                    pg[:], lhs, attnT[:, 2 * cp : 2 * cp + 2, :],
                    start=(cp == 0), stop=(cp == CT // 2 - 1), perf_mode=DR,
                )
            nc.scalar.activation(
                gxa_all[:, t * BL : (t + 1) * BL], pg[:], AF.Identity,
                scale=1.0 / (AT2_SCALE * W2_SCALE),
            )
            fn = section_done.pop(t + 1, None)
            if fn is not None:
                fn()

    nc.vector.tensor_add(gxfn[:], gxa_all[:, 2 * W : 3 * W], gx1[:, 2 * W : 3 * W])
    rhn = actp.tile([128, W], F32, tag="rhn")
    nc.vector.tensor_mul(rhn[:], r_all[:], gh[:, 2 * W : 3 * W])
    n_in = actp.tile([128, W], F32, tag="n_in")
    nc.vector.tensor_add(n_in[:], gxfn[:], rhn[:])
    n_all = actp.tile([128, W], F32, tag="n_all")
    nc.scalar.activation(n_all[:], n_in[:], AF.Tanh)
    hmn = actp.tile([128, W], F32, tag="hmn")
    nc.vector.tensor_sub(hmn[:], hTf_sb[:], n_all[:])
    zh = actp.tile([128, W], F32, tag="zh")
    nc.vector.tensor_mul(zh[:], z_all[:], hmn[:])
    h1T_all = actp.tile([128, W], F32, tag="h1T_all")
    nc.vector.tensor_add(h1T_all[:], n_all[:], zh[:])
    for ht in range(HT):
        ph = ms_ps.tile([BL, 128], F32, tag="ms")
        nc.tensor.transpose(
            ph[:], h1T_all[:, ht * BL : (ht + 1) * BL], identf[:]
        )
        if ht % 2 == 0:
            nc.vector.tensor_copy(h1nat[:, 128 * ht : 128 * (ht + 1)], ph[:])
        else:
            nc.scalar.activation(
                h1nat[:, 128 * ht : 128 * (ht + 1)], ph[:], AF.Identity
            )
        if ht == HT // 2 - 1:
            nc.sync.dma_start(
                d["d_h1"][:, 0 : HID // 2], h1nat[:, 0 : HID // 2]
            )
    nc.sync.dma_start(d["d_h1"][:, HID // 2 :], h1nat[:, HID // 2 :])
    stack.close()


_NC_CACHE = None


def _get_program():
    global _NC_CACHE
    if _NC_CACHE is None:
        _NC_CACHE = build_program()
    return _NC_CACHE


def _ptile(x, np_dtype):
    """[T*128, rest...] -> [128, T*rest] with partition dim first."""
    x = np.asarray(x, np.float32)
    t = x.shape[0] // 128
    out = x.reshape(t, 128, -1).transpose(1, 0, 2).reshape(128, -1)
    return np.ascontiguousarray(out.astype(np_dtype))


def make_in_maps(inputs):
    """Host-side prep: shard batch across cores, transpose/fuse weights."""
    f = lambda x: np.ascontiguousarray(np.asarray(x, dtype=np.float32))
    bf = lambda x: np.ascontiguousarray(np.asarray(x, dtype=np.float32).astype(BF16NP))
    input_ = f(inputs["input"])
    hidden = f(inputs["hidden"])
    context = f(inputs["context"])
    mask = np.asarray(inputs["context_mask"])
    Wq, bq = f(inputs["Wq"]), f(inputs["bq"])
    Wk, bk = f(inputs["Wk"]), f(inputs["bk"])
    Wl = f(inputs["Wl"])
    We, be = f(inputs["We"]), f(inputs["be"])
    Wa, ba = f(inputs["Wa"]), f(inputs["ba"])
    W_ih, W_hh = f(inputs["W_ih"]), f(inputs["W_hh"])
    b_ih, b_hh = f(inputs["b_ih"]), f(inputs["b_hh"])

    wlrep = np.broadcast_to(
        Wl[0].reshape(AT, 128).T[:, :, None], (128, AT, 128)
    ).reshape(128, AT * 128)

    def wgroups_of(WT, np_dtype):
        # [1024, H] -> per group g: [:, 256g:256(g+1)] as [128, 8, 256]
        H = WT.shape[1]
        g = WT.reshape(8, 128, H).transpose(1, 0, 2).reshape(128, 8, H // 256, 256)
        return np.ascontiguousarray(
            g.transpose(2, 0, 1, 3).astype(np_dtype)  # [NG, 128, 8, 256]
        )

    wq_g = wgroups_of(Wq.T.astype(np.float32), BF16NP)
    w1_g = wgroups_of((W_ih @ We).T, BF16NP)
    whh_g = wgroups_of(W_hh.T, BF16NP)
    wgall = np.concatenate(
        [wq_g.reshape(4, 128, -1), w1_g.reshape(12, 128, -1),
         whh_g.reshape(12, 128, -1)], axis=0
    )
    w2_g = wgroups_of(((W_ih @ Wa).T * W2_SCALE).astype(np.float32), FP8NP)
    shared = {
        "wk8": _ptile(Wk.T * WK_SCALE, FP8NP).reshape(128, CT, ATT),
        "wgall": np.ascontiguousarray(wgall),
        "w2g": np.ascontiguousarray(w2_g),
        "wlrep": np.ascontiguousarray(wlrep.astype(BF16NP)),
        "bqk": _ptile((bq + bk).reshape(AT * 128, 1), np.float32),
        "bx": _ptile((W_ih @ (be + ba) + b_ih).reshape(H3, 1), np.float32),
        "bhh": _ptile(b_hh.reshape(H3, 1), np.float32),
        "identf": np.eye(128, dtype=np.float32),
        "ones1": np.ones((1, 128), BF16NP),
        "one1b": np.ones((1, 1), BF16NP),
        "one32b": np.full((1, 1), AT2_SCALE, BF16NP),
    }
    pen = np.where(mask, np.float32(-1e18), np.float32(0.0)).astype(BF16NP)
    inT = np.ascontiguousarray(input_.T)
    hT = np.ascontiguousarray(hidden.T)

    in_maps = []
    for k in range(NCORES):
        sl = slice(k * BL, (k + 1) * BL)
        blk = context[sl]
        in_maps.append(
            {
                "ctxb": np.ascontiguousarray(
                    blk.reshape(BL, ST, 128, CTX).transpose(0, 2, 1, 3).astype(BF16NP)
                ),
                "ctxT8": np.ascontiguousarray(
                    blk.transpose(0, 2, 1).reshape(BL, CT, 128, S)
                    .transpose(0, 2, 1, 3).astype(FP8NP)
                ),
                "pen": np.ascontiguousarray(pen[sl].reshape(1, BL * S)),
                "inT": _ptile(inT[:, sl], BF16NP),
                "hT": _ptile(hT[:, sl], BF16NP),
                "hTf": _ptile(hT[:, sl], np.float32),
                **shared,
            }
        )
    return in_maps


def kernel(**inputs):
    nc = _get_program()
    in_maps = make_in_maps(inputs)
    res = run_bass_kernel_spmd(nc, in_maps, core_ids=list(range(NCORES)))
    hidden1 = np.concatenate([res.results[k]["h1"] for k in range(NCORES)], axis=0)
    attn = np.concatenate([res.results[k]["attn"] for k in range(NCORES)], axis=0)
    return (hidden1, attn)


# revision 19
# speedup vs baseline: 1.0284x; 1.0284x over previous
"""Trainium2 Bass kernel for an attention-augmented GRU cell (CGRUCell).

Reference computation (per batch row):
    cache   = context @ Wk.T + bk                  # [S, A]
    q       = hidden @ Wq.T + bq                   # [A]
    logits  = tanh(q + cache) @ Wl[0] + bl         # [S]
    logits  = where(mask, -1e18, logits)
    w       = softmax(logits)                      # [S]
    attn    = w @ context                          # [CTX]
    x       = input @ We.T + be + attn @ Wa.T + ba
    gx      = x @ W_ih.T + b_ih ; gh = hidden @ W_hh.T + b_hh
    r, z    = sigmoid(gx_r + gh_r), sigmoid(gx_z + gh_z)
    n       = tanh(gx_n + r * gh_n)
    hidden1 = (1 - z) * n + z * hidden
Outputs: (hidden1, attn)

Strategy: data-parallel over batch on 8 NeuronCores (8 rows each). The
dominant work is the [S,CTX]@[CTX,A] key projection; it runs on the
TensorEngine in fp8e4 DoubleRow mode (two contraction tiles per pass,
2x bf16 rate) off a host-pre-transposed fp8 copy of context, with Wk
host-scaled by 64 and the 1/64 folded into the tanh activation's
scale. The softmax reduction over the attention dim rides on bf16
matmuls against a 128-replicated Wl with the q/bk bias fused into the
tanh Activation op; the attention-value matvec contracts a bf16
natural-layout copy of context. The GRU algebra is reassociated
(W1 = W_ih@We, W2 = W_ih@Wa) so all of it except attn @ W2.T is
computed from the raw inputs by weight-stationary matmuls pipelined
through the batch rows. Each row's softmax/attention tail is emitted
interleaved into the next row's cache matmuls so the in-order PE never
idles on DVE/ACT latency. All host-staged tensors are laid out so
every DMA lands with contiguous per-partition chunks (strided-element
DMAs cost ~100x).
"""

import sys

if "/opt/trn_rl_repo" not in sys.path:
    sys.path.insert(0, "/opt/trn_rl_repo")

import ml_dtypes
import numpy as np

import concourse.bass as bass
import concourse.tile as tile
from concourse import bacc, mybir
from concourse.bass_utils import run_bass_kernel_spmd

NCORES = 8
B, S, IN, HID, CTX, ATT = 64, 1024, 1024, 1024, 1024, 1024
BL = B // NCORES          # batch rows per core
H3 = 3 * HID
AT, CT, HT, H3T = ATT // 128, CTX // 128, HID // 128, H3 // 128  # 8,8,8,24
ST = S // 128
F32 = mybir.dt.float32
BF16 = mybir.dt.bfloat16
FP8 = mybir.dt.float8e4
AX = mybir.AxisListType
AF = mybir.ActivationFunctionType
DR = mybir.MatmulPerfMode.DoubleRow
BF16NP = ml_dtypes.bfloat16
FP8NP = ml_dtypes.float8_e4m3
WK_SCALE = 64.0
AT2_SCALE = 32.0
W2_SCALE = 64.0


def build_program():
    nc = bacc.Bacc("TRN2", target_bir_lowering=False, debug=False, num_devices=NCORES)

    d_ctxb = nc.dram_tensor("ctxb", [BL, 128, ST, CTX], BF16, kind="ExternalInput").ap()
    d_ctxT8 = nc.dram_tensor("ctxT8", [BL, 128, CT, S], FP8, kind="ExternalInput").ap()
    d_pen = nc.dram_tensor("pen", [1, BL * S], BF16, kind="ExternalInput").ap()
    d_wlrep = nc.dram_tensor("wlrep", [128, AT * 128], BF16, kind="ExternalInput").ap()
    d_ones1 = nc.dram_tensor("ones1", [1, 128], BF16, kind="ExternalInput").ap()
    d_wk8 = nc.dram_tensor("wk8", [128, CT, ATT], FP8, kind="ExternalInput").ap()
    d_wgall = nc.dram_tensor(
        "wgall", [28, 128, HT * 2 * 128], BF16, kind="ExternalInput"
    ).ap()
    d_w2g = nc.dram_tensor(
        "w2g", [12, 128, CT, 2 * 128], FP8, kind="ExternalInput"
    ).ap()
    d_hT = nc.dram_tensor("hT", [128, HT * BL], BF16, kind="ExternalInput").ap()
    d_inT = nc.dram_tensor("inT", [128, HT * BL], BF16, kind="ExternalInput").ap()
    d_identf = nc.dram_tensor("identf", [128, 128], F32, kind="ExternalInput").ap()
    d_one1b = nc.dram_tensor("one1b", [1, 1], BF16, kind="ExternalInput").ap()
    d_hTf = nc.dram_tensor("hTf", [128, HT * BL], F32, kind="ExternalInput").ap()
    d_bqk = nc.dram_tensor("bqk", [128, AT], F32, kind="ExternalInput").ap()
    d_bx = nc.dram_tensor("bx", [128, H3T], F32, kind="ExternalInput").ap()
    d_bhh = nc.dram_tensor("bhh", [128, H3T], F32, kind="ExternalInput").ap()

    d_h1 = nc.dram_tensor("h1", [BL, HID], F32, kind="ExternalOutput").ap()
    d_attn = nc.dram_tensor("attn", [BL, CTX], F32, kind="ExternalOutput").ap()

    with tile.TileContext(nc) as tc:
        _emit(tc, locals())
    nc.compile()
    return nc


def _emit(tc, d):
    from contextlib import ExitStack

    nc = tc.nc
    CP = CT // 2  # contraction-tile pairs for DoubleRow

    stack = ExitStack()
    pool = lambda *a, **k: stack.enter_context(tc.tile_pool(*a, **k))
    cst = pool(name="cst", bufs=1)
    actp = pool(name="actp", bufs=1)
    wkp = pool(name="wkp", bufs=1)
    wstream = pool(name="wstream", bufs=6)
    natp = pool(name="natp", bufs=3)
    nat8p = pool(name="nat8p", bufs=2)
    tanhp = pool(name="tanhp", bufs=6)
    expp = pool(name="expp", bufs=2)
    arowp = pool(name="arowp", bufs=3)
    ecolp = pool(name="ecolp", bufs=2)
    smallp = pool(name="smallp", bufs=6)
    w2p = pool(name="w2p", bufs=5)

    # PSUM pools: 8 banks total (pc 4 + pl 2 + shared scratch 2)
    pc_ps = pool(name="pc_ps", bufs=4, space="PSUM")
    pl_ps = pool(name="pl_ps", bufs=2, space="PSUM")
    ms_ps = pool(name="ms_ps", bufs=2, space="PSUM")

    # ---- urgent loads first, spread across the three DMA queues:
    # gpsimd: nat8(0); scalar: wk then small consts then natb(0);
    # sync: hT then the pipelined weight-group stream.
    nat8_0 = nat8p.tile([128, CT, S], FP8, tag="nat8", name="nat8")
    wk_sb = wkp.tile([128, CT, ATT], FP8, tag="wk")
    for c2 in range(CT // 2):
        sl2 = slice(2 * c2, 2 * c2 + 2)
        nc.gpsimd.dma_start(nat8_0[:, sl2, :], d["d_ctxT8"][0][:, sl2, :])
        nc.scalar.dma_start(wk_sb[:, sl2, :], d["d_wk8"][:, sl2, :])
    hT_sb = actp.tile([128, HT * BL], BF16, tag="hT")
    nc.sync.dma_start(hT_sb[:], d["d_hT"][:])

    loads = {}

    def preload(b):
        natb = natp.tile([128, ST, CTX], BF16, tag="natb", name="natb")
        nc.scalar.dma_start(natb[:], d["d_ctxb"][b])
        nat8 = nat8p.tile([128, CT, S], FP8, tag="nat8", name="nat8")
        nc.gpsimd.dma_start(nat8[:], d["d_ctxT8"][b])
        loads[b] = (natb, nat8)

    # ---- small constants on the scalar queue, most-urgent first ----
    bqk_sb = cst.tile([128, AT], F32, tag="bqk")
    nc.scalar.dma_start(bqk_sb[:], d["d_bqk"][:])
    wlrep_sb = cst.tile([128, AT * 128], BF16, tag="wlrep")
    nc.scalar.dma_start(wlrep_sb[:], d["d_wlrep"][:])
    ones1 = cst.tile([1, 128], BF16, tag="ones1")
    nc.scalar.dma_start(ones1[:], d["d_ones1"][:])
    one1b = cst.tile([1, 1], BF16, tag="one1b")
    nc.scalar.dma_start(one1b[:], d["d_one1b"][:])
    pen_sb = cst.tile([1, BL * S], BF16, tag="pen")
    nc.scalar.dma_start(pen_sb[:], d["d_pen"][:])
    inT_sb = actp.tile([128, HT * BL], BF16, tag="inT")
    nc.scalar.dma_start(inT_sb[:], d["d_inT"][:])

    # natb(0) next (first needed at a==4 of row 0)
    natb_0 = natp.tile([128, ST, CTX], BF16, tag="natb", name="natb")
    nc.scalar.dma_start(natb_0[:], d["d_ctxb"][0])
    loads[0] = (natb_0, nat8_0)

    # tail-only constants: tiles declared here, DMAs deferred to row 1
    identf = cst.tile([128, 128], F32, tag="identf")
    bx_sb = cst.tile([128, H3T], F32, tag="bx")
    bhh_sb = cst.tile([128, H3T], F32, tag="bhh")
    hTf_sb = actp.tile([128, HT * BL], F32, tag="hTf")

    def load_tail_consts():
        nc.gpsimd.dma_start(identf[:], d["d_identf"][:])
        nc.gpsimd.dma_start(bx_sb[:], d["d_bx"][:])
        nc.gpsimd.dma_start(bhh_sb[:], d["d_bhh"][:])
        nc.gpsimd.dma_start(hTf_sb[:], d["d_hTf"][:])

    # ---- weight-stationary GEMMs: qeff = Wq@hT + (bq+bk); gx1 = W1@inT + bx;
    # gh = Whh@hT + bhh. DMA and matmul emission are pipelined (DMA runs two
    # groups ahead) and spread through the batch rows so the many small
    # LDWEIGHTS never starve the cache matmul stream.
    qeff = actp.tile([128, AT * BL], F32, tag="qeff")
    gx1 = actp.tile([128, H3T * BL], F32, tag="gx1")
    gh = actp.tile([128, H3T * BL], F32, tag="gh")
    TG = 2  # output tiles per weight DMA

    wspecs = [(qeff, bqk_sb, hT_sb, t0) for t0 in range(0, AT, TG)]
    wspecs += [(gx1, bx_sb, inT_sb, t0) for t0 in range(0, H3T, TG)]
    wspecs += [(gh, bhh_sb, hT_sb, t0) for t0 in range(0, H3T, TG)]
    wt_tiles = {}
    wcnt = {"dma": 0, "mm": 0}

    def dma_wgroup():
        k = wcnt["dma"]
        if k >= len(wspecs) or k - wcnt["mm"] >= 5:
            return
        wcnt["dma"] += 1
        wt = wstream.tile([128, HT * TG * 128], BF16, tag="ws", name="wt")
        eng = nc.sync if k % 2 == 0 else nc.gpsimd
        eng.dma_start(wt[:], d["d_wgall"][k])
        wt_tiles[k] = wt

    def mm_wgroup():
        k = wcnt["mm"]
        if k >= len(wspecs):
            return
        wcnt["mm"] += 1
        dst, bias_sb, rhs, t0 = wspecs[k]
        wt = wt_tiles.pop(k)
        for tl in range(TG):
            t = t0 + tl
            pg = ms_ps.tile([128, BL], F32, tag="ms")
            for j in range(HT):
                lhs = wt[:, j * TG * 128 + tl * 128 : j * TG * 128 + (tl + 1) * 128]
                nc.tensor.matmul(
                    pg[:], lhs, rhs[:, j * BL : (j + 1) * BL],
                    start=(j == 0), stop=(j == HT - 1),
                )
            nc.scalar.activation(
                dst[:, t * BL : (t + 1) * BL], pg[:], AF.Identity,
                bias=bias_sb[:, t : t + 1],
            )

    # ---- main attention loop over local batch rows ----
    # Batch row b's softmax/attention tail is deferred and emitted at
    # checkpoints inside row b+1's cache-matmul loop so the in-order PE
    # always has dense matmul work while DVE/ACT chase the softmax
    # dependency chain.
    sums = actp.tile([128, BL], F32, tag="sums")
    recip = actp.tile([128, BL], F32, tag="recip")
    attnT = actp.tile([128, CT, BL], FP8, tag="attnT")
    deferred = []  # closures carrying batch b-1's softmax/attn chunks
    w2tiles = []

    def make_chunks(b, natb, plb0, plb1):
        state = {}

        def run1():  # max + exp straight off the broadcast-logits psum
            mx2 = smallp.tile([128, 2], F32, tag="mx2")
            nc.vector.reduce_max(mx2[:, 0:1], plb0[:], axis=AX.X)
            nc.vector.reduce_max(mx2[:, 1:2], plb1[:], axis=AX.X)
            nmx = smallp.tile([128, 1], F32, tag="nmx")
            nc.vector.reduce_max(nmx[:], mx2[:], axis=AX.X, negate=True)
            acc2 = smallp.tile([128, 2], F32, tag="acc2")
            etile = expp.tile([128, S], BF16, tag="exp")
            nc.scalar.activation(
                etile[:, 0:512], plb0[:], AF.Exp, bias=nmx[:], accum_out=acc2[:, 0:1]
            )
            nc.scalar.activation(
                etile[:, 512:1024], plb1[:], AF.Exp, bias=nmx[:], accum_out=acc2[:, 1:2]
            )
            nc.vector.tensor_add(sums[:, b : b + 1], acc2[:, 0:1], acc2[:, 1:2])
            nc.vector.reciprocal(recip[:, b : b + 1], sums[:, b : b + 1])
            state["etile"] = etile

        def run2():  # exp row -> column layout for the matvec
            etile = state["etile"]
            pe = ms_ps.tile([128, ST], F32, tag="ms")
            for st in range(ST):
                nc.tensor.matmul(
                    pe[:, st : st + 1],
                    etile[0:1, 128 * st : 128 * (st + 1)],
                    one1b[:],
                    start=True, stop=True,
                )
            ecol = ecolp.tile([128, ST], BF16, tag="ecol")
            nc.vector.tensor_copy(ecol[:], pe[:])
            state["ecol"] = ecol

        def run3():  # attention values + normalized output row
            ecol = state["ecol"]
            arow = arowp.tile([1, CTX], F32, tag="arow")
            for cg in range(2):
                pav = ms_ps.tile([1, 512], F32, tag="ms")
                for st in range(ST):
                    nc.tensor.matmul(
                        pav[:], ecol[:, st : st + 1],
                        natb[:, st, 512 * cg : 512 * (cg + 1)],
                        start=(st == 0), stop=(st == ST - 1),
                    )
                nc.vector.tensor_copy(arow[:, 512 * cg : 512 * (cg + 1)], pav[:])
            an = arowp.tile([1, CTX], F32, tag="arow")
            nc.vector.tensor_scalar_mul(an[:], arow[:], recip[0:1, b : b + 1])
            nc.sync.dma_start(d["d_attn"][b : b + 1, :], an[:])
            arowb = arowp.tile([1, CTX], BF16, tag="arow")
            nc.vector.tensor_scalar_mul(arowb[:], an[:], AT2_SCALE)
            state["arowb"] = arowb

        def run4():  # attnT columns for the W2 matmul
            arowb = state["arowb"]
            pat = ms_ps.tile([128, CT], F32, tag="ms")
            for c in range(CT):
                nc.tensor.matmul(
                    pat[:, c : c + 1],
                    arowb[0:1, 128 * c : 128 * (c + 1)],
                    one1b[:],
                    start=True, stop=True,
                )
            nc.vector.tensor_copy(attnT[:, :, b], pat[:])

        return [run1, run2, run3, run4]

    for _ in range(4):
        dma_wgroup()
    mm_wgroup()  # qeff tiles 0-1, needed by row 0's first tanh
    finish_prev = None
    for b in range(BL):
        natb, nat8 = loads.pop(b)
        if b == 1:
            load_tail_consts()

        if b == BL - 1:
            # pre-issue the tail's W2 stream on the now-idle SWDGE queue so
            # the gxa matmuls aren't DMA-paced after the last batch row
            for t0 in range(0, H3T, TG):
                wt2 = w2p.tile([128, CT, TG * 128], FP8, tag="w2s", name="wt2")
                nc.gpsimd.dma_start(wt2[:], d["d_w2g"][t0 // TG])
                w2tiles.append(wt2)

        # cache matmul (fp8 DoubleRow) + tanh + broadcast-logits reduction.
        # The replicated Wl stationary operand makes the Wl-contraction emit
        # logits replicated across all 128 partitions, ready for softmax.
        # pl matmuls for a-1 are emitted after the cache matmuls of a so the
        # PE never waits on the tanh ACT drain.
        plb0 = pl_ps.tile([128, 512], F32, tag="pl")
        plb1 = pl_ps.tile([128, 512], F32, tag="pl")
        pending = []

        def emit_pl(th0, th1, a, plb0=plb0, plb1=plb1):
            lhs = wlrep_sb[:, a * 128 : (a + 1) * 128]
            nc.tensor.matmul(plb0[:], lhs, th0[:], start=(a == 0), stop=False)
            nc.tensor.matmul(plb1[:], lhs, th1[:], start=(a == 0), stop=False)

        for a in range(AT):
            pc0 = pc_ps.tile([128, 512], F32, tag="pc")
            pc1 = pc_ps.tile([128, 512], F32, tag="pc")
            for cp in range(CP):
                lhs = wk_sb[:, 2 * cp : 2 * cp + 2, 128 * a : 128 * (a + 1)]
                nc.tensor.matmul(
                    pc0[:], lhs, nat8[:, 2 * cp : 2 * cp + 2, 0:512],
                    start=(cp == 0), stop=(cp == CP - 1), perf_mode=DR,
                )
                nc.tensor.matmul(
                    pc1[:], lhs, nat8[:, 2 * cp : 2 * cp + 2, 512:1024],
                    start=(cp == 0), stop=(cp == CP - 1), perf_mode=DR,
                )
            if a == 0 and finish_prev is not None:
                finish_prev()  # prev batch's pl(7) + penalty matmuls
            if deferred:
                if a == 1:
                    deferred[0]()  # DVE/ACT only: max+exp off the pl psum
                elif a == 2:
                    deferred[1]()
                elif a == 4:
                    deferred[2]()
                elif a == 6:
                    deferred[3]()
            if a == 1 and b + 1 < BL:
                preload(b + 1)
            if len(pending) >= 2:
                emit_pl(*pending.pop(0))
            if a < 4:
                dma_wgroup()
                if (b >= 2 or wcnt["mm"] < 4) and wcnt["mm"] < len(wspecs) - 3:
                    mm_wgroup()
            th0 = tanhp.tile([128, 512], BF16, tag="tanh")
            th1 = tanhp.tile([128, 512], BF16, tag="tanh")
            qcol = qeff[:, a * BL + b : a * BL + b + 1]
            nc.scalar.activation(
                th0[:], pc0[:], AF.Tanh, bias=qcol, scale=1.0 / WK_SCALE
            )
            nc.scalar.activation(
                th1[:], pc1[:], AF.Tanh, bias=qcol, scale=1.0 / WK_SCALE
            )
            pending.append((th0, th1, a))

        def finish_prev(
            pending=pending, emit_pl=emit_pl, plb0=plb0, plb1=plb1, b=b
        ):
            for p in pending:
                emit_pl(*p)
            # fold the additive mask penalties into the broadcast logits
            nc.tensor.matmul(
                plb0[:], ones1[:], pen_sb[0:1, b * S : b * S + 512], start=False, stop=True
            )
            nc.tensor.matmul(
                plb1[:], ones1[:], pen_sb[0:1, b * S + 512 : b * S + 1024], start=False, stop=True
            )

        deferred = make_chunks(b, natb, plb0, plb1)

    finish_prev()  # flush last batch row
    for fn in deferred:
        fn()
        dma_wgroup()
        mm_wgroup()  # reserved weight groups keep the PE fed during softmax
    while wcnt["mm"] < len(wspecs):
        dma_wgroup()
        mm_wgroup()

    # ---- tail: gxa = W2 @ attnT (attnT already normalized, fp8 x32; W2
    # fp8 x64 -> psum carries 2048x, folded out in the Identity copy). The
    # gate math is emitted per r/z/n section as soon as that section's gxa
    # groups land, so DVE/ACT overlap the LDWEIGHTS-paced gxa stream.
    W = HT * BL  # 64
    gxa_all = actp.tile([128, H3T * BL], F32, tag="gxa_all")
    h1nat = actp.tile([BL, HID], F32, tag="h1nat")
    r_all = actp.tile([128, W], F32, tag="r_all")
    z_all = actp.tile([128, W], F32, tag="z_all")
    gxfn = actp.tile([128, W], F32, tag="gxfn")

    def gates_r():
        rz = actp.tile([128, W], F32, tag="rz")
        nc.vector.tensor_add(rz[:], gxa_all[:, 0:W], gx1[:, 0:W])
        nc.vector.tensor_add(rz[:], rz[:], gh[:, 0:W])
        nc.scalar.activation(r_all[:], rz[:], AF.Sigmoid)

    def gates_z():
        rz = actp.tile([128, W], F32, tag="rz")
        nc.vector.tensor_add(rz[:], gxa_all[:, W : 2 * W], gx1[:, W : 2 * W])
        nc.vector.tensor_add(rz[:], rz[:], gh[:, W : 2 * W])
        nc.scalar.activation(z_all[:], rz[:], AF.Sigmoid)

    section_done = {8: gates_r, 16: gates_z}
    for t0 in range(0, H3T, TG):
        wt2 = w2tiles[t0 // TG]
        for tl in range(TG):
            t = t0 + tl
            pg = ms_ps.tile([128, BL], F32, tag="ms")
            for cp in range(CT // 2):
                lhs = wt2[:, 2 * cp : 2 * cp + 2, tl * 128 : (tl + 1) * 128]
                nc.tensor.matmul(
                    pg[:], lhs, attnT[:, 2 * cp : 2 * cp + 2, :],
                    start=(cp == 0), stop=(cp == CT // 2 - 1), perf_mode=DR,
                )
            nc.scalar.activation(
                gxa_all[:, t * BL : (t + 1) * BL], pg[:], AF.Identity,
                scale=1.0 / (AT2_SCALE * W2_SCALE),
            )
            fn = section_done.pop(t + 1, None)
            if fn is not None:
                fn()

    nc.vector.tensor_add(gxfn[:], gxa_all[:, 2 * W : 3 * W], gx1[:, 2 * W : 3 * W])
    rhn = actp.tile([128, W], F32, tag="rhn")
    nc.vector.tensor_mul(rhn[:], r_all[:], gh[:, 2 * W : 3 * W])
    n_in = actp.tile([128, W], F32, tag="n_in")
    nc.vector.tensor_add(n_in[:], gxfn[:], rhn[:])
    n_all = actp.tile([128, W], F32, tag="n_all")
    nc.scalar.activation(n_all[:], n_in[:], AF.Tanh)
    hmn = actp.tile([128, W], F32, tag="hmn")
    nc.vector.tensor_sub(hmn[:], hTf_sb[:], n_all[:])
    zh = actp.tile([128, W], F32, tag="zh")
    nc.vector.tensor_mul(zh[:], z_all[:], hmn[:])
    h1T_all = actp.tile([128, W], F32, tag="h1T_all")
    nc.vector.tensor_add(h1T_all[:], n_all[:], zh[:])
    for ht in range(HT):
        ph = ms_ps.tile([BL, 128], F32, tag="ms")
        nc.tensor.transpose(
            ph[:], h1T_all[:, ht * BL : (ht + 1) * BL], identf[:]
        )
        nc.vector.tensor_copy(h1nat[:, 128 * ht : 128 * (ht + 1)], ph[:])
        if ht == HT // 2 - 1:
            nc.sync.dma_start(
                d["d_h1"][:, 0 : HID // 2], h1nat[:, 0 : HID // 2]
            )
    nc.sync.dma_start(d["d_h1"][:, HID // 2 :], h1nat[:, HID // 2 :])
    stack.close()


_NC_CACHE = None


def _get_program():
    global _NC_CACHE
    if _NC_CACHE is None:
        _NC_CACHE = build_program()
    return _NC_CACHE


def _ptile(x, np_dtype):
    """[T*128, rest...] -> [128, T*rest] with partition dim first."""
    x = np.asarray(x, np.float32)
    t = x.shape[0] // 128
    out = x.reshape(t, 128, -1).transpose(1, 0, 2).reshape(128, -1)
    return np.ascontiguousarray(out.astype(np_dtype))


def make_in_maps(inputs):
    """Host-side prep: shard batch across cores, transpose/fuse weights."""
    f = lambda x: np.ascontiguousarray(np.asarray(x, dtype=np.float32))
    bf = lambda x: np.ascontiguousarray(np.asarray(x, dtype=np.float32).astype(BF16NP))
    input_ = f(inputs["input"])
    hidden = f(inputs["hidden"])
    context = f(inputs["context"])
    mask = np.asarray(inputs["context_mask"])
    Wq, bq = f(inputs["Wq"]), f(inputs["bq"])
    Wk, bk = f(inputs["Wk"]), f(inputs["bk"])
    Wl = f(inputs["Wl"])
    We, be = f(inputs["We"]), f(inputs["be"])
    Wa, ba = f(inputs["Wa"]), f(inputs["ba"])
    W_ih, W_hh = f(inputs["W_ih"]), f(inputs["W_hh"])
    b_ih, b_hh = f(inputs["b_ih"]), f(inputs["b_hh"])

    wlrep = np.broadcast_to(
        Wl[0].reshape(AT, 128).T[:, :, None], (128, AT, 128)
    ).reshape(128, AT * 128)

    def wgroups_of(WT, np_dtype):
        # [1024, H] -> per group g: [:, 256g:256(g+1)] as [128, 8, 256]
        H = WT.shape[1]
        g = WT.reshape(8, 128, H).transpose(1, 0, 2).reshape(128, 8, H // 256, 256)
        return np.ascontiguousarray(
            g.transpose(2, 0, 1, 3).astype(np_dtype)  # [NG, 128, 8, 256]
        )

    wq_g = wgroups_of(Wq.T.astype(np.float32), BF16NP)
    w1_g = wgroups_of((W_ih @ We).T, BF16NP)
    whh_g = wgroups_of(W_hh.T, BF16NP)
    wgall = np.concatenate(
        [wq_g.reshape(4, 128, -1), w1_g.reshape(12, 128, -1),
         whh_g.reshape(12, 128, -1)], axis=0
    )
    w2_g = wgroups_of(((W_ih @ Wa).T * W2_SCALE).astype(np.float32), FP8NP)
    shared = {
        "wk8": _ptile(Wk.T * WK_SCALE, FP8NP).reshape(128, CT, ATT),
        "wgall": np.ascontiguousarray(wgall),
        "w2g": np.ascontiguousarray(w2_g),
        "wlrep": np.ascontiguousarray(wlrep.astype(BF16NP)),
        "bqk": _ptile((bq + bk).reshape(AT * 128, 1), np.float32),
        "bx": _ptile((W_ih @ (be + ba) + b_ih).reshape(H3, 1), np.float32),
        "bhh": _ptile(b_hh.reshape(H3, 1), np.float32),
        "identf": np.eye(128, dtype=np.float32),
        "ones1": np.ones((1, 128), BF16NP),
        "one1b": np.ones((1, 1), BF16NP),
    }
    pen = np.where(mask, np.float32(-1e18), np.float32(0.0)).astype(BF16NP)
    inT = np.ascontiguousarray(input_.T)
    hT = np.ascontiguousarray(hidden.T)

    in_maps = []
    for k in range(NCORES):
        sl = slice(k * BL, (k + 1) * BL)
        blk = context[sl]
        in_maps.append(
            {
                "ctxb": np.ascontiguousarray(
                    blk.reshape(BL, ST, 128, CTX).transpose(0, 2, 1, 3).astype(BF16NP)
                ),
                "ctxT8": np.ascontiguousarray(
                    blk.transpose(0, 2, 1).reshape(BL, CT, 128, S)
                    .transpose(0, 2, 1, 3).astype(FP8NP)
                ),
                "pen": np.ascontiguousarray(pen[sl].reshape(1, BL * S)),
                "inT": _ptile(inT[:, sl], BF16NP),
                "hT": _ptile(hT[:, sl], BF16NP),
                "hTf": _ptile(hT[:, sl], np.float32),
                **shared,
            }
        )
    return in_maps


def kernel(**inputs):
    nc = _get_program()
    in_maps = make_in_maps(inputs)
    res = run_bass_kernel_spmd(nc, in_maps, core_ids=list(range(NCORES)))
    hidden1 = np.concatenate([res.results[k]["h1"] for k in range(NCORES)], axis=0)
    attn = np.concatenate([res.results[k]["attn"] for k in range(NCORES)], axis=0)
    return (hidden1, attn)


# revision 20
# speedup vs baseline: 1.0310x; 1.0025x over previous
"""Trainium2 Bass kernel for an attention-augmented GRU cell (CGRUCell).

Reference computation (per batch row):
    cache   = context @ Wk.T + bk                  # [S, A]
    q       = hidden @ Wq.T + bq                   # [A]
    logits  = tanh(q + cache) @ Wl[0] + bl         # [S]
    logits  = where(mask, -1e18, logits)
    w       = softmax(logits)                      # [S]
    attn    = w @ context                          # [CTX]
    x       = input @ We.T + be + attn @ Wa.T + ba
    gx      = x @ W_ih.T + b_ih ; gh = hidden @ W_hh.T + b_hh
    r, z    = sigmoid(gx_r + gh_r), sigmoid(gx_z + gh_z)
    n       = tanh(gx_n + r * gh_n)
    hidden1 = (1 - z) * n + z * hidden
Outputs: (hidden1, attn)

Strategy: data-parallel over batch on 8 NeuronCores (8 rows each). The
dominant work is the [S,CTX]@[CTX,A] key projection; it runs on the
TensorEngine in fp8e4 DoubleRow mode (two contraction tiles per pass,
2x bf16 rate) off a host-pre-transposed fp8 copy of context, with Wk
host-scaled by 64 and the 1/64 folded into the tanh activation's
scale. The softmax reduction over the attention dim rides on bf16
matmuls against a 128-replicated Wl with the q/bk bias fused into the
tanh Activation op; the attention-value matvec contracts a bf16
natural-layout copy of context. The GRU algebra is reassociated
(W1 = W_ih@We, W2 = W_ih@Wa) so all of it except attn @ W2.T is
computed from the raw inputs by weight-stationary matmuls pipelined
through the batch rows. Each row's softmax/attention tail is emitted
interleaved into the next row's cache matmuls so the in-order PE never
idles on DVE/ACT latency. All host-staged tensors are laid out so
every DMA lands with contiguous per-partition chunks (strided-element
DMAs cost ~100x).
"""

import sys

if "/opt/trn_rl_repo" not in sys.path:
    sys.path.insert(0, "/opt/trn_rl_repo")

import ml_dtypes
import numpy as np

import concourse.bass as bass
import concourse.tile as tile
from concourse import bacc, mybir
from concourse.bass_utils import run_bass_kernel_spmd

NCORES = 8
B, S, IN, HID, CTX, ATT = 64, 1024, 1024, 1024, 1024, 1024
BL = B // NCORES          # batch rows per core
H3 = 3 * HID
AT, CT, HT, H3T = ATT // 128, CTX // 128, HID // 128, H3 // 128  # 8,8,8,24
ST = S // 128
F32 = mybir.dt.float32
BF16 = mybir.dt.bfloat16
FP8 = mybir.dt.float8e4
AX = mybir.AxisListType
AF = mybir.ActivationFunctionType
DR = mybir.MatmulPerfMode.DoubleRow
BF16NP = ml_dtypes.bfloat16
FP8NP = ml_dtypes.float8_e4m3
WK_SCALE = 64.0
AT2_SCALE = 32.0
W2_SCALE = 64.0


def build_program():
    nc = bacc.Bacc("TRN2", target_bir_lowering=False, debug=False, num_devices=NCORES)

    d_ctxb = nc.dram_tensor("ctxb", [BL, 128, ST, CTX], BF16, kind="ExternalInput").ap()
    d_ctxT8 = nc.dram_tensor("ctxT8", [BL, 128, CT, S], FP8, kind="ExternalInput").ap()
    d_pen = nc.dram_tensor("pen", [1, BL * S], BF16, kind="ExternalInput").ap()
    d_wlrep = nc.dram_tensor("wlrep", [128, AT * 128], BF16, kind="ExternalInput").ap()
    d_ones1 = nc.dram_tensor("ones1", [1, 128], BF16, kind="ExternalInput").ap()
    d_wk8 = nc.dram_tensor("wk8", [128, CT, ATT], FP8, kind="ExternalInput").ap()
    d_wgall = nc.dram_tensor(
        "wgall", [28, 128, HT * 2 * 128], BF16, kind="ExternalInput"
    ).ap()
    d_w2g = nc.dram_tensor(
        "w2g", [12, 128, CT, 2 * 128], FP8, kind="ExternalInput"
    ).ap()
    d_hT = nc.dram_tensor("hT", [128, HT * BL], BF16, kind="ExternalInput").ap()
    d_inT = nc.dram_tensor("inT", [128, HT * BL], BF16, kind="ExternalInput").ap()
    d_identf = nc.dram_tensor("identf", [128, 128], F32, kind="ExternalInput").ap()
    d_one1b = nc.dram_tensor("one1b", [1, 1], BF16, kind="ExternalInput").ap()
    d_hTf = nc.dram_tensor("hTf", [128, HT * BL], F32, kind="ExternalInput").ap()
    d_bqk = nc.dram_tensor("bqk", [128, AT], F32, kind="ExternalInput").ap()
    d_bx = nc.dram_tensor("bx", [128, H3T], F32, kind="ExternalInput").ap()
    d_bhh = nc.dram_tensor("bhh", [128, H3T], F32, kind="ExternalInput").ap()

    d_h1 = nc.dram_tensor("h1", [BL, HID], F32, kind="ExternalOutput").ap()
    d_attn = nc.dram_tensor("attn", [BL, CTX], F32, kind="ExternalOutput").ap()

    with tile.TileContext(nc) as tc:
        _emit(tc, locals())
    nc.compile()
    return nc


def _emit(tc, d):
    from contextlib import ExitStack

    nc = tc.nc
    CP = CT // 2  # contraction-tile pairs for DoubleRow

    stack = ExitStack()
    pool = lambda *a, **k: stack.enter_context(tc.tile_pool(*a, **k))
    cst = pool(name="cst", bufs=1)
    actp = pool(name="actp", bufs=1)
    wkp = pool(name="wkp", bufs=1)
    wstream = pool(name="wstream", bufs=6)
    natp = pool(name="natp", bufs=3)
    nat8p = pool(name="nat8p", bufs=2)
    tanhp = pool(name="tanhp", bufs=6)
    expp = pool(name="expp", bufs=2)
    arowp = pool(name="arowp", bufs=3)
    ecolp = pool(name="ecolp", bufs=2)
    smallp = pool(name="smallp", bufs=6)
    w2p = pool(name="w2p", bufs=5)

    # PSUM pools: 8 banks total (pc 4 + pl 2 + shared scratch 2)
    pc_ps = pool(name="pc_ps", bufs=4, space="PSUM")
    pl_ps = pool(name="pl_ps", bufs=2, space="PSUM")
    ms_ps = pool(name="ms_ps", bufs=2, space="PSUM")

    # ---- urgent loads first, spread across the three DMA queues:
    # gpsimd: nat8(0); scalar: wk then small consts then natb(0);
    # sync: hT then the pipelined weight-group stream.
    nat8_0 = nat8p.tile([128, CT, S], FP8, tag="nat8", name="nat8")
    wk_sb = wkp.tile([128, CT, ATT], FP8, tag="wk")
    for c2 in range(CT // 2):
        sl2 = slice(2 * c2, 2 * c2 + 2)
        nc.gpsimd.dma_start(nat8_0[:, sl2, :], d["d_ctxT8"][0][:, sl2, :])
        nc.scalar.dma_start(wk_sb[:, sl2, :], d["d_wk8"][:, sl2, :])
    hT_sb = actp.tile([128, HT * BL], BF16, tag="hT")
    nc.sync.dma_start(hT_sb[:], d["d_hT"][:])

    loads = {}

    def preload(b):
        natb = natp.tile([128, ST, CTX], BF16, tag="natb", name="natb")
        nc.scalar.dma_start(natb[:], d["d_ctxb"][b])
        nat8 = nat8p.tile([128, CT, S], FP8, tag="nat8", name="nat8")
        nc.gpsimd.dma_start(nat8[:], d["d_ctxT8"][b])
        loads[b] = (natb, nat8)

    # ---- small constants on the scalar queue, most-urgent first ----
    bqk_sb = cst.tile([128, AT], F32, tag="bqk")
    nc.scalar.dma_start(bqk_sb[:], d["d_bqk"][:])
    wlrep_sb = cst.tile([128, AT * 128], BF16, tag="wlrep")
    nc.scalar.dma_start(wlrep_sb[:], d["d_wlrep"][:])
    ones1 = cst.tile([1, 128], BF16, tag="ones1")
    nc.scalar.dma_start(ones1[:], d["d_ones1"][:])
    one1b = cst.tile([1, 1], BF16, tag="one1b")
    nc.scalar.dma_start(one1b[:], d["d_one1b"][:])
    pen_sb = cst.tile([1, BL * S], BF16, tag="pen")
    nc.scalar.dma_start(pen_sb[:], d["d_pen"][:])
    inT_sb = actp.tile([128, HT * BL], BF16, tag="inT")
    nc.scalar.dma_start(inT_sb[:], d["d_inT"][:])

    # natb(0) next (first needed at a==4 of row 0)
    natb_0 = natp.tile([128, ST, CTX], BF16, tag="natb", name="natb")
    nc.scalar.dma_start(natb_0[:], d["d_ctxb"][0])
    loads[0] = (natb_0, nat8_0)

    # tail-only constants: tiles declared here, DMAs deferred to row 1
    identf = cst.tile([128, 128], F32, tag="identf")
    bx_sb = cst.tile([128, H3T], F32, tag="bx")
    bhh_sb = cst.tile([128, H3T], F32, tag="bhh")
    hTf_sb = actp.tile([128, HT * BL], F32, tag="hTf")

    def load_tail_consts():
        nc.gpsimd.dma_start(identf[:], d["d_identf"][:])
        nc.gpsimd.dma_start(bx_sb[:], d["d_bx"][:])
        nc.gpsimd.dma_start(bhh_sb[:], d["d_bhh"][:])
        nc.gpsimd.dma_start(hTf_sb[:], d["d_hTf"][:])

    # ---- weight-stationary GEMMs: qeff = Wq@hT + (bq+bk); gx1 = W1@inT + bx;
    # gh = Whh@hT + bhh. DMA and matmul emission are pipelined (DMA runs two
    # groups ahead) and spread through the batch rows so the many small
    # LDWEIGHTS never starve the cache matmul stream.
    qeff = actp.tile([128, AT * BL], F32, tag="qeff")
    gx1 = actp.tile([128, H3T * BL], F32, tag="gx1")
    gh = actp.tile([128, H3T * BL], F32, tag="gh")
    TG = 2  # output tiles per weight DMA

    wspecs = [(qeff, bqk_sb, hT_sb, t0) for t0 in range(0, AT, TG)]
    wspecs += [(gx1, bx_sb, inT_sb, t0) for t0 in range(0, H3T, TG)]
    wspecs += [(gh, bhh_sb, hT_sb, t0) for t0 in range(0, H3T, TG)]
    wt_tiles = {}
    wcnt = {"dma": 0, "mm": 0}

    def dma_wgroup():
        k = wcnt["dma"]
        if k >= len(wspecs) or k - wcnt["mm"] >= 5:
            return
        wcnt["dma"] += 1
        wt = wstream.tile([128, HT * TG * 128], BF16, tag="ws", name="wt")
        eng = nc.sync if k % 2 == 0 else nc.gpsimd
        eng.dma_start(wt[:], d["d_wgall"][k])
        wt_tiles[k] = wt

    def mm_wgroup():
        k = wcnt["mm"]
        if k >= len(wspecs):
            return
        wcnt["mm"] += 1
        dst, bias_sb, rhs, t0 = wspecs[k]
        wt = wt_tiles.pop(k)
        for tl in range(TG):
            t = t0 + tl
            pg = ms_ps.tile([128, BL], F32, tag="ms")
            for j in range(HT):
                lhs = wt[:, j * TG * 128 + tl * 128 : j * TG * 128 + (tl + 1) * 128]
                nc.tensor.matmul(
                    pg[:], lhs, rhs[:, j * BL : (j + 1) * BL],
                    start=(j == 0), stop=(j == HT - 1),
                )
            nc.scalar.activation(
                dst[:, t * BL : (t + 1) * BL], pg[:], AF.Identity,
                bias=bias_sb[:, t : t + 1],
            )

    # ---- main attention loop over local batch rows ----
    # Batch row b's softmax/attention tail is deferred and emitted at
    # checkpoints inside row b+1's cache-matmul loop so the in-order PE
    # always has dense matmul work while DVE/ACT chase the softmax
    # dependency chain.
    sums = actp.tile([128, BL], F32, tag="sums")
    recip = actp.tile([128, BL], F32, tag="recip")
    attnT = actp.tile([128, CT, BL], FP8, tag="attnT")
    deferred = []  # closures carrying batch b-1's softmax/attn chunks
    w2tiles = []

    def make_chunks(b, natb, plb0, plb1):
        state = {}

        def run1():  # max + exp straight off the broadcast-logits psum
            mx2 = smallp.tile([128, 2], F32, tag="mx2")
            nc.vector.reduce_max(mx2[:, 0:1], plb0[:], axis=AX.X)
            nc.vector.reduce_max(mx2[:, 1:2], plb1[:], axis=AX.X)
            nmx = smallp.tile([128, 1], F32, tag="nmx")
            nc.vector.reduce_max(nmx[:], mx2[:], axis=AX.X, negate=True)
            acc2 = smallp.tile([128, 2], F32, tag="acc2")
            etile = expp.tile([128, S], BF16, tag="exp")
            nc.scalar.activation(
                etile[:, 0:512], plb0[:], AF.Exp, bias=nmx[:], accum_out=acc2[:, 0:1]
            )
            nc.scalar.activation(
                etile[:, 512:1024], plb1[:], AF.Exp, bias=nmx[:], accum_out=acc2[:, 1:2]
            )
            nc.vector.tensor_add(sums[:, b : b + 1], acc2[:, 0:1], acc2[:, 1:2])
            nc.vector.reciprocal(recip[:, b : b + 1], sums[:, b : b + 1])
            r32 = smallp.tile([128, 1], F32, tag="r32")
            nc.vector.tensor_scalar_mul(r32[:], recip[:, b : b + 1], AT2_SCALE)
            state["r32"] = r32
            state["etile"] = etile

        def run2():  # exp row -> column layout for the matvec
            etile = state["etile"]
            pe = ms_ps.tile([128, ST], F32, tag="ms")
            for st in range(ST):
                nc.tensor.matmul(
                    pe[:, st : st + 1],
                    etile[0:1, 128 * st : 128 * (st + 1)],
                    one1b[:],
                    start=True, stop=True,
                )
            ecol = ecolp.tile([128, ST], BF16, tag="ecol")
            nc.vector.tensor_copy(ecol[:], pe[:])
            state["ecol"] = ecol

        def run3():  # attention values + normalized output row
            ecol = state["ecol"]
            arow = arowp.tile([1, CTX], F32, tag="arow")
            for cg in range(2):
                pav = ms_ps.tile([1, 512], F32, tag="ms")
                for st in range(ST):
                    nc.tensor.matmul(
                        pav[:], ecol[:, st : st + 1],
                        natb[:, st, 512 * cg : 512 * (cg + 1)],
                        start=(st == 0), stop=(st == ST - 1),
                    )
                nc.vector.tensor_copy(arow[:, 512 * cg : 512 * (cg + 1)], pav[:])
            arowb = arowp.tile([1, CTX], BF16, tag="arow")
            nc.vector.tensor_scalar_mul(arowb[:], arow[:], state["r32"][0:1, :])
            state["arowb"] = arowb
            state["arow"] = arow

        def run4():  # attnT columns for the W2 matmul
            arowb = state["arowb"]
            pat = ms_ps.tile([128, CT], F32, tag="ms")
            for c in range(CT):
                nc.tensor.matmul(
                    pat[:, c : c + 1],
                    arowb[0:1, 128 * c : 128 * (c + 1)],
                    one1b[:],
                    start=True, stop=True,
                )
            nc.vector.tensor_copy(attnT[:, :, b], pat[:])
            an = arowp.tile([1, CTX], F32, tag="arow")
            nc.vector.tensor_scalar_mul(an[:], state["arow"], recip[0:1, b : b + 1])
            nc.sync.dma_start(d["d_attn"][b : b + 1, :], an[:])

        return [run1, run2, run3, run4]

    for _ in range(4):
        dma_wgroup()
    mm_wgroup()  # qeff tiles 0-1, needed by row 0's first tanh
    finish_prev = None
    for b in range(BL):
        natb, nat8 = loads.pop(b)
        if b == 1:
            load_tail_consts()

        if b == BL - 1:
            # pre-issue the tail's W2 stream on the now-idle SWDGE queue so
            # the gxa matmuls aren't DMA-paced after the last batch row
            for t0 in range(0, H3T, TG):
                wt2 = w2p.tile([128, CT, TG * 128], FP8, tag="w2s", name="wt2")
                nc.gpsimd.dma_start(wt2[:], d["d_w2g"][t0 // TG])
                w2tiles.append(wt2)

        # cache matmul (fp8 DoubleRow) + tanh + broadcast-logits reduction.
        # The replicated Wl stationary operand makes the Wl-contraction emit
        # logits replicated across all 128 partitions, ready for softmax.
        # pl matmuls for a-1 are emitted after the cache matmuls of a so the
        # PE never waits on the tanh ACT drain.
        plb0 = pl_ps.tile([128, 512], F32, tag="pl")
        plb1 = pl_ps.tile([128, 512], F32, tag="pl")
        pending = []

        def emit_pl(th0, th1, a, plb0=plb0, plb1=plb1):
            lhs = wlrep_sb[:, a * 128 : (a + 1) * 128]
            nc.tensor.matmul(plb0[:], lhs, th0[:], start=(a == 0), stop=False)
            nc.tensor.matmul(plb1[:], lhs, th1[:], start=(a == 0), stop=False)

        for a in range(AT):
            pc0 = pc_ps.tile([128, 512], F32, tag="pc")
            pc1 = pc_ps.tile([128, 512], F32, tag="pc")
            for cp in range(CP):
                lhs = wk_sb[:, 2 * cp : 2 * cp + 2, 128 * a : 128 * (a + 1)]
                nc.tensor.matmul(
                    pc0[:], lhs, nat8[:, 2 * cp : 2 * cp + 2, 0:512],
                    start=(cp == 0), stop=(cp == CP - 1), perf_mode=DR,
                )
                nc.tensor.matmul(
                    pc1[:], lhs, nat8[:, 2 * cp : 2 * cp + 2, 512:1024],
                    start=(cp == 0), stop=(cp == CP - 1), perf_mode=DR,
                )
            if a == 0 and finish_prev is not None:
                finish_prev()  # prev batch's pl(7) + penalty matmuls
            if deferred:
                if a == 1:
                    deferred[0]()  # DVE/ACT only: max+exp off the pl psum
                elif a == 2:
                    deferred[1]()
                elif a == 4:
                    deferred[2]()
                elif a == 6:
                    deferred[3]()
            if a == 1 and b + 1 < BL:
                preload(b + 1)
            if len(pending) >= (2 if b + 1 < BL else 1):
                emit_pl(*pending.pop(0))
            if a < 4:
                dma_wgroup()
                if (b >= 2 or wcnt["mm"] < 4) and wcnt["mm"] < len(wspecs) - 3:
                    mm_wgroup()
            th0 = tanhp.tile([128, 512], BF16, tag="tanh")
            th1 = tanhp.tile([128, 512], BF16, tag="tanh")
            qcol = qeff[:, a * BL + b : a * BL + b + 1]
            nc.scalar.activation(
                th0[:], pc0[:], AF.Tanh, bias=qcol, scale=1.0 / WK_SCALE
            )
            nc.scalar.activation(
                th1[:], pc1[:], AF.Tanh, bias=qcol, scale=1.0 / WK_SCALE
            )
            pending.append((th0, th1, a))

        def finish_prev(
            pending=pending, emit_pl=emit_pl, plb0=plb0, plb1=plb1, b=b
        ):
            for p in pending:
                emit_pl(*p)
            # fold the additive mask penalties into the broadcast logits
            nc.tensor.matmul(
                plb0[:], ones1[:], pen_sb[0:1, b * S : b * S + 512], start=False, stop=True
            )
            nc.tensor.matmul(
                plb1[:], ones1[:], pen_sb[0:1, b * S + 512 : b * S + 1024], start=False, stop=True
            )

        deferred = make_chunks(b, natb, plb0, plb1)

    finish_prev()  # flush last batch row
    for fn in deferred:
        fn()
        dma_wgroup()
        mm_wgroup()  # reserved weight groups keep the PE fed during softmax
    while wcnt["mm"] < len(wspecs):
        dma_wgroup()
        mm_wgroup()

    # ---- tail: gxa = W2 @ attnT (attnT already normalized, fp8 x32; W2
    # fp8 x64 -> psum carries 2048x, folded out in the Identity copy). The
    # gate math is emitted per r/z/n section as soon as that section's gxa
    # groups land, so DVE/ACT overlap the LDWEIGHTS-paced gxa stream.
    W = HT * BL  # 64
    gxa_all = actp.tile([128, H3T * BL], F32, tag="gxa_all")
    h1nat = actp.tile([BL, HID], F32, tag="h1nat")
    r_all = actp.tile([128, W], F32, tag="r_all")
    z_all = actp.tile([128, W], F32, tag="z_all")
    gxfn = actp.tile([128, W], F32, tag="gxfn")

    def gates_r():
        rz = actp.tile([128, W], F32, tag="rz")
        nc.vector.tensor_add(rz[:], gxa_all[:, 0:W], gx1[:, 0:W])
        nc.vector.tensor_add(rz[:], rz[:], gh[:, 0:W])
        nc.scalar.activation(r_all[:], rz[:], AF.Sigmoid)

    def gates_z():
        rz = actp.tile([128, W], F32, tag="rz")
        nc.vector.tensor_add(rz[:], gxa_all[:, W : 2 * W], gx1[:, W : 2 * W])
        nc.vector.tensor_add(rz[:], rz[:], gh[:, W : 2 * W])
        nc.scalar.activation(z_all[:], rz[:], AF.Sigmoid)

    section_done = {8: gates_r, 16: gates_z}
    for t0 in range(0, H3T, TG):
        wt2 = w2tiles[t0 // TG]
        for tl in range(TG):
            t = t0 + tl
            pg = ms_ps.tile([128, BL], F32, tag="ms")
            for cp in range(CT // 2):
                lhs = wt2[:, 2 * cp : 2 * cp + 2, tl * 128 : (tl + 1) * 128]
                nc.tensor.matmul(
                    pg[:], lhs, attnT[:, 2 * cp : 2 * cp + 2, :],
                    start=(cp == 0), stop=(cp == CT // 2 - 1), perf_mode=DR,
                )
            nc.scalar.activation(
                gxa_all[:, t * BL : (t + 1) * BL], pg[:], AF.Identity,
                scale=1.0 / (AT2_SCALE * W2_SCALE),
            )
            fn = section_done.pop(t + 1, None)
            if fn is not None:
                fn()

    nc.vector.tensor_add(gxfn[:], gxa_all[:, 2 * W : 3 * W], gx1[:, 2 * W : 3 * W])
    rhn = actp.tile([128, W], F32, tag="rhn")
    nc.vector.tensor_mul(rhn[:], r_all[:], gh[:, 2 * W : 3 * W])
    n_in = actp.tile([128, W], F32, tag="n_in")
    nc.vector.tensor_add(n_in[:], gxfn[:], rhn[:])
    n_all = actp.tile([128, W], F32, tag="n_all")
    nc.scalar.activation(n_all[:], n_in[:], AF.Tanh)
    hmn = actp.tile([128, W], F32, tag="hmn")
    nc.vector.tensor_sub(hmn[:], hTf_sb[:], n_all[:])
    zh = actp.tile([128, W], F32, tag="zh")
    nc.vector.tensor_mul(zh[:], z_all[:], hmn[:])
    h1T_all = actp.tile([128, W], F32, tag="h1T_all")
    nc.vector.tensor_add(h1T_all[:], n_all[:], zh[:])
    for ht in range(HT):
        ph = ms_ps.tile([BL, 128], F32, tag="ms")
        nc.tensor.transpose(
            ph[:], h1T_all[:, ht * BL : (ht + 1) * BL], identf[:]
        )
        nc.vector.tensor_copy(h1nat[:, 128 * ht : 128 * (ht + 1)], ph[:])
        if ht == HT // 2 - 1:
            nc.sync.dma_start(
                d["d_h1"][:, 0 : HID // 2], h1nat[:, 0 : HID // 2]
            )
    nc.sync.dma_start(d["d_h1"][:, HID // 2 :], h1nat[:, HID // 2 :])
    stack.close()


_NC_CACHE = None


def _get_program():
    global _NC_CACHE
    if _NC_CACHE is None:
        _NC_CACHE = build_program()
    return _NC_CACHE


def _ptile(x, np_dtype):
    """[T*128, rest...] -> [128, T*rest] with partition dim first."""
    x = np.asarray(x, np.float32)
    t = x.shape[0] // 128
    out = x.reshape(t, 128, -1).transpose(1, 0, 2).reshape(128, -1)
    return np.ascontiguousarray(out.astype(np_dtype))


def make_in_maps(inputs):
    """Host-side prep: shard batch across cores, transpose/fuse weights."""
    f = lambda x: np.ascontiguousarray(np.asarray(x, dtype=np.float32))
    bf = lambda x: np.ascontiguousarray(np.asarray(x, dtype=np.float32).astype(BF16NP))
    input_ = f(inputs["input"])
    hidden = f(inputs["hidden"])
    context = f(inputs["context"])
    mask = np.asarray(inputs["context_mask"])
    Wq, bq = f(inputs["Wq"]), f(inputs["bq"])
    Wk, bk = f(inputs["Wk"]), f(inputs["bk"])
    Wl = f(inputs["Wl"])
    We, be = f(inputs["We"]), f(inputs["be"])
    Wa, ba = f(inputs["Wa"]), f(inputs["ba"])
    W_ih, W_hh = f(inputs["W_ih"]), f(inputs["W_hh"])
    b_ih, b_hh = f(inputs["b_ih"]), f(inputs["b_hh"])

    wlrep = np.broadcast_to(
        Wl[0].reshape(AT, 128).T[:, :, None], (128, AT, 128)
    ).reshape(128, AT * 128)

    def wgroups_of(WT, np_dtype):
        # [1024, H] -> per group g: [:, 256g:256(g+1)] as [128, 8, 256]
        H = WT.shape[1]
        g = WT.reshape(8, 128, H).transpose(1, 0, 2).reshape(128, 8, H // 256, 256)
        return np.ascontiguousarray(
            g.transpose(2, 0, 1, 3).astype(np_dtype)  # [NG, 128, 8, 256]
        )

    wq_g = wgroups_of(Wq.T.astype(np.float32), BF16NP)
    w1_g = wgroups_of((W_ih @ We).T, BF16NP)
    whh_g = wgroups_of(W_hh.T, BF16NP)
    wgall = np.concatenate(
        [wq_g.reshape(4, 128, -1), w1_g.reshape(12, 128, -1),
         whh_g.reshape(12, 128, -1)], axis=0
    )
    w2_g = wgroups_of(((W_ih @ Wa).T * W2_SCALE).astype(np.float32), FP8NP)
    shared = {
        "wk8": _ptile(Wk.T * WK_SCALE, FP8NP).reshape(128, CT, ATT),
        "wgall": np.ascontiguousarray(wgall),
        "w2g": np.ascontiguousarray(w2_g),
        "wlrep": np.ascontiguousarray(wlrep.astype(BF16NP)),
        "bqk": _ptile((bq + bk).reshape(AT * 128, 1), np.float32),
        "bx": _ptile((W_ih @ (be + ba) + b_ih).reshape(H3, 1), np.float32),
        "bhh": _ptile(b_hh.reshape(H3, 1), np.float32),
        "identf": np.eye(128, dtype=np.float32),
        "ones1": np.ones((1, 128), BF16NP),
        "one1b": np.ones((1, 1), BF16NP),
    }
    pen = np.where(mask, np.float32(-1e18), np.float32(0.0)).astype(BF16NP)
    inT = np.ascontiguousarray(input_.T)
    hT = np.ascontiguousarray(hidden.T)

    in_maps = []
    for k in range(NCORES):
        sl = slice(k * BL, (k + 1) * BL)
        blk = context[sl]
        in_maps.append(
            {
                "ctxb": np.ascontiguousarray(
                    blk.reshape(BL, ST, 128, CTX).transpose(0, 2, 1, 3).astype(BF16NP)
                ),
                "ctxT8": np.ascontiguousarray(
                    blk.transpose(0, 2, 1).reshape(BL, CT, 128, S)
                    .transpose(0, 2, 1, 3).astype(FP8NP)
                ),
                "pen": np.ascontiguousarray(pen[sl].reshape(1, BL * S)),
                "inT": _ptile(inT[:, sl], BF16NP),
                "hT": _ptile(hT[:, sl], BF16NP),
                "hTf": _ptile(hT[:, sl], np.float32),
                **shared,
            }
        )
    return in_maps


def kernel(**inputs):
    nc = _get_program()
    in_maps = make_in_maps(inputs)
    res = run_bass_kernel_spmd(nc, in_maps, core_ids=list(range(NCORES)))
    hidden1 = np.concatenate([res.results[k]["h1"] for k in range(NCORES)], axis=0)
    attn = np.concatenate([res.results[k]["attn"] for k in range(NCORES)], axis=0)
    return (hidden1, attn)


# revision 21
# speedup vs baseline: 1.0409x; 1.0097x over previous
"""Trainium2 Bass kernel for an attention-augmented GRU cell (CGRUCell).

Reference computation (per batch row):
    cache   = context @ Wk.T + bk                  # [S, A]
    q       = hidden @ Wq.T + bq                   # [A]
    logits  = tanh(q + cache) @ Wl[0] + bl         # [S]
    logits  = where(mask, -1e18, logits)
    w       = softmax(logits)                      # [S]
    attn    = w @ context                          # [CTX]
    x       = input @ We.T + be + attn @ Wa.T + ba
    gx      = x @ W_ih.T + b_ih ; gh = hidden @ W_hh.T + b_hh
    r, z    = sigmoid(gx_r + gh_r), sigmoid(gx_z + gh_z)
    n       = tanh(gx_n + r * gh_n)
    hidden1 = (1 - z) * n + z * hidden
Outputs: (hidden1, attn)

Strategy: data-parallel over batch on 8 NeuronCores (8 rows each). The
dominant work is the [S,CTX]@[CTX,A] key projection; it runs on the
TensorEngine in fp8e4 DoubleRow mode (two contraction tiles per pass,
2x bf16 rate) off a host-pre-transposed fp8 copy of context, with Wk
host-scaled by 64 and the 1/64 folded into the tanh activation's
scale. The softmax reduction over the attention dim rides on bf16
matmuls against a 128-replicated Wl with the q/bk bias fused into the
tanh Activation op; the attention-value matvec contracts a bf16
natural-layout copy of context. The GRU algebra is reassociated
(W1 = W_ih@We, W2 = W_ih@Wa) so all of it except attn @ W2.T is
computed from the raw inputs by weight-stationary matmuls pipelined
through the batch rows. Each row's softmax/attention tail is emitted
interleaved into the next row's cache matmuls so the in-order PE never
idles on DVE/ACT latency. All host-staged tensors are laid out so
every DMA lands with contiguous per-partition chunks (strided-element
DMAs cost ~100x).
"""

import sys

if "/opt/trn_rl_repo" not in sys.path:
    sys.path.insert(0, "/opt/trn_rl_repo")

import ml_dtypes
import numpy as np

import concourse.bass as bass
import concourse.tile as tile
from concourse import bacc, mybir
from concourse.bass_utils import run_bass_kernel_spmd

NCORES = 8
B, S, IN, HID, CTX, ATT = 64, 1024, 1024, 1024, 1024, 1024
BL = B // NCORES          # batch rows per core
H3 = 3 * HID
AT, CT, HT, H3T = ATT // 128, CTX // 128, HID // 128, H3 // 128  # 8,8,8,24
ST = S // 128
F32 = mybir.dt.float32
BF16 = mybir.dt.bfloat16
FP8 = mybir.dt.float8e4
AX = mybir.AxisListType
AF = mybir.ActivationFunctionType
DR = mybir.MatmulPerfMode.DoubleRow
BF16NP = ml_dtypes.bfloat16
FP8NP = ml_dtypes.float8_e4m3
WK_SCALE = 64.0
AT2_SCALE = 32.0
W2_SCALE = 64.0


def build_program():
    nc = bacc.Bacc("TRN2", target_bir_lowering=False, debug=False, num_devices=NCORES)

    d_ctxb = nc.dram_tensor("ctxb", [BL, 128, ST, CTX], BF16, kind="ExternalInput").ap()
    d_ctxT8 = nc.dram_tensor("ctxT8", [BL, 128, CT, S], FP8, kind="ExternalInput").ap()
    d_pen = nc.dram_tensor("pen", [1, BL * S], BF16, kind="ExternalInput").ap()
    d_wlrep = nc.dram_tensor("wlrep", [128, AT * 128], BF16, kind="ExternalInput").ap()
    d_ones1 = nc.dram_tensor("ones1", [1, 128], BF16, kind="ExternalInput").ap()
    d_wk8 = nc.dram_tensor("wk8", [128, CT, ATT], FP8, kind="ExternalInput").ap()
    d_wgall = nc.dram_tensor(
        "wgall", [28, 128, HT * 2 * 128], BF16, kind="ExternalInput"
    ).ap()
    d_w2g = nc.dram_tensor(
        "w2g", [12, 128, CT, 2 * 128], FP8, kind="ExternalInput"
    ).ap()
    d_hT = nc.dram_tensor("hT", [128, HT * BL], BF16, kind="ExternalInput").ap()
    d_inT = nc.dram_tensor("inT", [128, HT * BL], BF16, kind="ExternalInput").ap()
    d_identf = nc.dram_tensor("identf", [128, 128], F32, kind="ExternalInput").ap()
    d_one1b = nc.dram_tensor("one1b", [1, 1], BF16, kind="ExternalInput").ap()
    d_hTf = nc.dram_tensor("hTf", [128, HT * BL], F32, kind="ExternalInput").ap()
    d_bqk = nc.dram_tensor("bqk", [128, AT], F32, kind="ExternalInput").ap()
    d_bx = nc.dram_tensor("bx", [128, H3T], F32, kind="ExternalInput").ap()
    d_bhh = nc.dram_tensor("bhh", [128, H3T], F32, kind="ExternalInput").ap()

    d_h1 = nc.dram_tensor("h1", [BL, HID], F32, kind="ExternalOutput").ap()
    d_attn = nc.dram_tensor("attn", [BL, CTX], F32, kind="ExternalOutput").ap()

    with tile.TileContext(nc) as tc:
        _emit(tc, locals())
    nc.compile()
    return nc


def _emit(tc, d):
    from contextlib import ExitStack

    nc = tc.nc
    CP = CT // 2  # contraction-tile pairs for DoubleRow

    stack = ExitStack()
    pool = lambda *a, **k: stack.enter_context(tc.tile_pool(*a, **k))
    cst = pool(name="cst", bufs=1)
    actp = pool(name="actp", bufs=1)
    wkp = pool(name="wkp", bufs=1)
    wstream = pool(name="wstream", bufs=6)
    natp = pool(name="natp", bufs=3)
    nat8p = pool(name="nat8p", bufs=2)
    tanhp = pool(name="tanhp", bufs=6)
    expp = pool(name="expp", bufs=2)
    arowp = pool(name="arowp", bufs=3)
    ecolp = pool(name="ecolp", bufs=2)
    smallp = pool(name="smallp", bufs=6)
    w2p = pool(name="w2p", bufs=5)

    # PSUM pools: 8 banks total (pc 4 + pl 2 + shared scratch 2)
    pc_ps = pool(name="pc_ps", bufs=4, space="PSUM")
    pl_ps = pool(name="pl_ps", bufs=2, space="PSUM")
    ms_ps = pool(name="ms_ps", bufs=2, space="PSUM")

    # ---- urgent loads first, spread across the three DMA queues:
    # gpsimd: nat8(0); scalar: wk then small consts then natb(0);
    # sync: hT then the pipelined weight-group stream.
    nat8_0 = nat8p.tile([128, CT, S], FP8, tag="nat8", name="nat8")
    wk_sb = wkp.tile([128, CT, ATT], FP8, tag="wk")
    for c2 in range(CT // 2):
        sl2 = slice(2 * c2, 2 * c2 + 2)
        nc.gpsimd.dma_start(nat8_0[:, sl2, :], d["d_ctxT8"][0][:, sl2, :])
        nc.scalar.dma_start(wk_sb[:, sl2, :], d["d_wk8"][:, sl2, :])
    hT_sb = actp.tile([128, HT * BL], BF16, tag="hT")
    nc.sync.dma_start(hT_sb[:], d["d_hT"][:])

    loads = {}

    def preload(b):
        natb = natp.tile([128, ST, CTX], BF16, tag="natb", name="natb")
        nc.scalar.dma_start(natb[:], d["d_ctxb"][b])
        nat8 = nat8p.tile([128, CT, S], FP8, tag="nat8", name="nat8")
        nc.gpsimd.dma_start(nat8[:], d["d_ctxT8"][b])
        loads[b] = (natb, nat8)

    # ---- small constants on the scalar queue, most-urgent first ----
    bqk_sb = cst.tile([128, AT], F32, tag="bqk")
    nc.scalar.dma_start(bqk_sb[:], d["d_bqk"][:])
    wlrep_sb = cst.tile([128, AT * 128], BF16, tag="wlrep")
    nc.scalar.dma_start(wlrep_sb[:], d["d_wlrep"][:])
    ones1 = cst.tile([1, 128], BF16, tag="ones1")
    nc.scalar.dma_start(ones1[:], d["d_ones1"][:])
    one1b = cst.tile([1, 1], BF16, tag="one1b")
    nc.scalar.dma_start(one1b[:], d["d_one1b"][:])
    pen_sb = cst.tile([1, BL * S], BF16, tag="pen")
    nc.scalar.dma_start(pen_sb[:], d["d_pen"][:])
    inT_sb = actp.tile([128, HT * BL], BF16, tag="inT")
    nc.scalar.dma_start(inT_sb[:], d["d_inT"][:])

    # natb(0) next (first needed at a==4 of row 0)
    natb_0 = natp.tile([128, ST, CTX], BF16, tag="natb", name="natb")
    nc.scalar.dma_start(natb_0[:], d["d_ctxb"][0])
    loads[0] = (natb_0, nat8_0)

    # tail-only constants: tiles declared here, DMAs deferred to row 1
    identf = cst.tile([128, 128], F32, tag="identf")
    bx_sb = cst.tile([128, H3T], F32, tag="bx")
    bhh_sb = cst.tile([128, H3T], F32, tag="bhh")
    hTf_sb = actp.tile([128, HT * BL], F32, tag="hTf")

    def load_tail_consts():
        nc.gpsimd.dma_start(identf[:], d["d_identf"][:])
        nc.gpsimd.dma_start(bx_sb[:], d["d_bx"][:])
        nc.gpsimd.dma_start(bhh_sb[:], d["d_bhh"][:])
        nc.gpsimd.dma_start(hTf_sb[:], d["d_hTf"][:])

    # ---- weight-stationary GEMMs: qeff = Wq@hT + (bq+bk); gx1 = W1@inT + bx;
    # gh = Whh@hT + bhh. DMA and matmul emission are pipelined (DMA runs two
    # groups ahead) and spread through the batch rows so the many small
    # LDWEIGHTS never starve the cache matmul stream.
    qeff = actp.tile([128, AT * BL], F32, tag="qeff")
    gx1 = actp.tile([128, H3T * BL], F32, tag="gx1")
    gh = actp.tile([128, H3T * BL], F32, tag="gh")
    TG = 2  # output tiles per weight DMA

    wspecs = [(qeff, bqk_sb, hT_sb, t0) for t0 in range(0, AT, TG)]
    wspecs += [(gx1, bx_sb, inT_sb, t0) for t0 in range(0, H3T, TG)]
    wspecs += [(gh, bhh_sb, hT_sb, t0) for t0 in range(0, H3T, TG)]
    wt_tiles = {}
    wcnt = {"dma": 0, "mm": 0}

    def dma_wgroup():
        k = wcnt["dma"]
        if k >= len(wspecs) or k - wcnt["mm"] >= 5:
            return
        wcnt["dma"] += 1
        wt = wstream.tile([128, HT * TG * 128], BF16, tag="ws", name="wt")
        eng = nc.sync if k % 2 == 0 else nc.gpsimd
        eng.dma_start(wt[:], d["d_wgall"][k])
        wt_tiles[k] = wt

    def mm_wgroup():
        k = wcnt["mm"]
        if k >= len(wspecs):
            return
        wcnt["mm"] += 1
        dst, bias_sb, rhs, t0 = wspecs[k]
        wt = wt_tiles.pop(k)
        for tl in range(TG):
            t = t0 + tl
            pg = ms_ps.tile([128, BL], F32, tag="ms")
            for j in range(HT):
                lhs = wt[:, j * TG * 128 + tl * 128 : j * TG * 128 + (tl + 1) * 128]
                nc.tensor.matmul(
                    pg[:], lhs, rhs[:, j * BL : (j + 1) * BL],
                    start=(j == 0), stop=(j == HT - 1),
                )
            nc.scalar.activation(
                dst[:, t * BL : (t + 1) * BL], pg[:], AF.Identity,
                bias=bias_sb[:, t : t + 1],
            )

    # ---- main attention loop over local batch rows ----
    # Batch row b's softmax/attention tail is deferred and emitted at
    # checkpoints inside row b+1's cache-matmul loop so the in-order PE
    # always has dense matmul work while DVE/ACT chase the softmax
    # dependency chain.
    sums = actp.tile([128, BL], F32, tag="sums")
    recip = actp.tile([128, BL], F32, tag="recip")
    attnT = actp.tile([128, CT, BL], FP8, tag="attnT")
    deferred = []  # closures carrying batch b-1's softmax/attn chunks
    w2tiles = []

    def make_chunks(b, natb, plb0, plb1):
        state = {}

        def run1():  # max + exp straight off the broadcast-logits psum
            mx2 = smallp.tile([128, 2], F32, tag="mx2")
            nc.vector.reduce_max(mx2[:, 0:1], plb0[:], axis=AX.X)
            nc.vector.reduce_max(mx2[:, 1:2], plb1[:], axis=AX.X)
            nmx = smallp.tile([128, 1], F32, tag="nmx")
            nc.vector.reduce_max(nmx[:], mx2[:], axis=AX.X, negate=True)
            acc2 = smallp.tile([128, 2], F32, tag="acc2")
            etile = expp.tile([128, S], BF16, tag="exp")
            nc.scalar.activation(
                etile[:, 0:512], plb0[:], AF.Exp, bias=nmx[:], accum_out=acc2[:, 0:1]
            )
            nc.scalar.activation(
                etile[:, 512:1024], plb1[:], AF.Exp, bias=nmx[:], accum_out=acc2[:, 1:2]
            )
            nc.vector.tensor_add(sums[:, b : b + 1], acc2[:, 0:1], acc2[:, 1:2])
            nc.vector.reciprocal(recip[:, b : b + 1], sums[:, b : b + 1])
            r32 = smallp.tile([128, 1], F32, tag="r32")
            nc.vector.tensor_scalar_mul(r32[:], recip[:, b : b + 1], AT2_SCALE)
            state["r32"] = r32
            state["etile"] = etile

        def run2():  # exp row -> column layout for the matvec
            etile = state["etile"]
            pe = ms_ps.tile([128, ST], F32, tag="ms")
            for st in range(ST):
                nc.tensor.matmul(
                    pe[:, st : st + 1],
                    etile[0:1, 128 * st : 128 * (st + 1)],
                    one1b[:],
                    start=True, stop=True,
                )
            ecol = ecolp.tile([128, ST], BF16, tag="ecol")
            nc.vector.tensor_copy(ecol[:], pe[:])
            state["ecol"] = ecol

        def run3():  # attention values + normalized output row
            ecol = state["ecol"]
            arow = arowp.tile([1, CTX], F32, tag="arow")
            for cg in range(2):
                pav = ms_ps.tile([1, 512], F32, tag="ms")
                for st in range(ST):
                    nc.tensor.matmul(
                        pav[:], ecol[:, st : st + 1],
                        natb[:, st, 512 * cg : 512 * (cg + 1)],
                        start=(st == 0), stop=(st == ST - 1),
                    )
                nc.vector.tensor_copy(arow[:, 512 * cg : 512 * (cg + 1)], pav[:])
            arowb = arowp.tile([1, CTX], BF16, tag="arow")
            nc.vector.tensor_scalar_mul(arowb[:], arow[:], state["r32"][0:1, :])
            state["arowb"] = arowb
            state["arow"] = arow

        def run4():  # attnT columns for the W2 matmul
            arowb = state["arowb"]
            pat = ms_ps.tile([128, CT], F32, tag="ms")
            for c in range(CT):
                nc.tensor.matmul(
                    pat[:, c : c + 1],
                    arowb[0:1, 128 * c : 128 * (c + 1)],
                    one1b[:],
                    start=True, stop=True,
                )
            nc.vector.tensor_copy(attnT[:, :, b], pat[:])
            an = arowp.tile([1, CTX], F32, tag="arow")
            nc.vector.tensor_scalar_mul(an[:], state["arow"], recip[0:1, b : b + 1])
            nc.sync.dma_start(d["d_attn"][b : b + 1, :], an[:])

        return [run1, run2, run3, run4]

    for _ in range(4):
        dma_wgroup()
    mm_wgroup()  # qeff tiles 0-1, needed by row 0's first tanh
    finish_prev = None
    for b in range(BL):
        natb, nat8 = loads.pop(b)
        if b == 1:
            load_tail_consts()

        if b == BL - 1:
            # pre-issue the tail's W2 stream on the now-idle SWDGE queue so
            # the gxa matmuls aren't DMA-paced after the last batch row
            for t0 in range(0, H3T, TG):
                wt2 = w2p.tile([128, CT, TG * 128], FP8, tag="w2s", name="wt2")
                nc.gpsimd.dma_start(wt2[:], d["d_w2g"][t0 // TG])
                w2tiles.append(wt2)

        # cache matmul (fp8 DoubleRow) + tanh + broadcast-logits reduction.
        # The replicated Wl stationary operand makes the Wl-contraction emit
        # logits replicated across all 128 partitions, ready for softmax.
        # pl matmuls for a-1 are emitted after the cache matmuls of a so the
        # PE never waits on the tanh ACT drain.
        plb0 = pl_ps.tile([128, 512], F32, tag="pl")
        plb1 = pl_ps.tile([128, 512], F32, tag="pl")
        pending = []

        def emit_pl(th0, th1, a, plb0=plb0, plb1=plb1):
            lhs = wlrep_sb[:, a * 128 : (a + 1) * 128]
            nc.tensor.matmul(plb0[:], lhs, th0[:], start=(a == 0), stop=False)
            nc.tensor.matmul(plb1[:], lhs, th1[:], start=(a == 0), stop=False)

        for a in range(AT):
            pc0 = pc_ps.tile([128, 512], F32, tag="pc")
            pc1 = pc_ps.tile([128, 512], F32, tag="pc")
            for cp in range(CP):
                lhs = wk_sb[:, 2 * cp : 2 * cp + 2, 128 * a : 128 * (a + 1)]
                nc.tensor.matmul(
                    pc0[:], lhs, nat8[:, 2 * cp : 2 * cp + 2, 0:512],
                    start=(cp == 0), stop=(cp == CP - 1), perf_mode=DR,
                )
                nc.tensor.matmul(
                    pc1[:], lhs, nat8[:, 2 * cp : 2 * cp + 2, 512:1024],
                    start=(cp == 0), stop=(cp == CP - 1), perf_mode=DR,
                )
            if a == 0 and finish_prev is not None:
                finish_prev()  # prev batch's pl(7) + penalty matmuls
            if deferred:
                if a == 1:
                    deferred[0]()  # DVE/ACT only: max+exp off the pl psum
                elif a == 2:
                    deferred[1]()
                elif a == 4:
                    deferred[2]()
                elif a == 6:
                    deferred[3]()
            if a == 1 and b + 1 < BL:
                preload(b + 1)
            if len(pending) >= (2 if b + 1 < BL else 1):
                emit_pl(*pending.pop(0))
            if a < 4:
                dma_wgroup()
                if (b >= 2 or wcnt["mm"] < 4) and wcnt["mm"] < len(wspecs) - 4:
                    mm_wgroup()
            th0 = tanhp.tile([128, 512], BF16, tag="tanh")
            th1 = tanhp.tile([128, 512], BF16, tag="tanh")
            qcol = qeff[:, a * BL + b : a * BL + b + 1]
            nc.scalar.activation(
                th0[:], pc0[:], AF.Tanh, bias=qcol, scale=1.0 / WK_SCALE
            )
            nc.scalar.activation(
                th1[:], pc1[:], AF.Tanh, bias=qcol, scale=1.0 / WK_SCALE
            )
            pending.append((th0, th1, a))

        def finish_prev(
            pending=pending, emit_pl=emit_pl, plb0=plb0, plb1=plb1, b=b
        ):
            for p in pending:
                emit_pl(*p)
            # fold the additive mask penalties into the broadcast logits
            nc.tensor.matmul(
                plb0[:], ones1[:], pen_sb[0:1, b * S : b * S + 512], start=False, stop=True
            )
            nc.tensor.matmul(
                plb1[:], ones1[:], pen_sb[0:1, b * S + 512 : b * S + 1024], start=False, stop=True
            )

        deferred = make_chunks(b, natb, plb0, plb1)

    finish_prev()  # flush last batch row
    for i, fn in enumerate(deferred):
        fn()
        dma_wgroup()
        mm_wgroup()  # reserved weight groups keep the PE fed during softmax
        if i == 0:
            dma_wgroup()
            mm_wgroup()
    while wcnt["mm"] < len(wspecs):
        dma_wgroup()
        mm_wgroup()

    # ---- tail: gxa = W2 @ attnT (attnT already normalized, fp8 x32; W2
    # fp8 x64 -> psum carries 2048x, folded out in the Identity copy). The
    # gate math is emitted per r/z/n section as soon as that section's gxa
    # groups land, so DVE/ACT overlap the LDWEIGHTS-paced gxa stream.
    W = HT * BL  # 64
    gxa_all = actp.tile([128, H3T * BL], F32, tag="gxa_all")
    h1nat = actp.tile([BL, HID], F32, tag="h1nat")
    r_all = actp.tile([128, W], F32, tag="r_all")
    z_all = actp.tile([128, W], F32, tag="z_all")
    gxfn = actp.tile([128, W], F32, tag="gxfn")

    def gates_r():
        rz = actp.tile([128, W], F32, tag="rz")
        nc.vector.tensor_add(rz[:], gxa_all[:, 0:W], gx1[:, 0:W])
        nc.vector.tensor_add(rz[:], rz[:], gh[:, 0:W])
        nc.scalar.activation(r_all[:], rz[:], AF.Sigmoid)

    def gates_z():
        rz = actp.tile([128, W], F32, tag="rz")
        nc.vector.tensor_add(rz[:], gxa_all[:, W : 2 * W], gx1[:, W : 2 * W])
        nc.vector.tensor_add(rz[:], rz[:], gh[:, W : 2 * W])
        nc.scalar.activation(z_all[:], rz[:], AF.Sigmoid)

    section_done = {8: gates_r, 16: gates_z}
    for t0 in range(0, H3T, TG):
        wt2 = w2tiles[t0 // TG]
        for tl in range(TG):
            t = t0 + tl
            pg = ms_ps.tile([128, BL], F32, tag="ms")
            for cp in range(CT // 2):
                lhs = wt2[:, 2 * cp : 2 * cp + 2, tl * 128 : (tl + 1) * 128]
                nc.tensor.matmul(
                    pg[:], lhs, attnT[:, 2 * cp : 2 * cp + 2, :],
                    start=(cp == 0), stop=(cp == CT // 2 - 1), perf_mode=DR,
                )
            nc.scalar.activation(
                gxa_all[:, t * BL : (t + 1) * BL], pg[:], AF.Identity,
                scale=1.0 / (AT2_SCALE * W2_SCALE),
            )
            fn = section_done.pop(t + 1, None)
            if fn is not None:
                fn()

    rhn = actp.tile([128, W], F32, tag="rhn")
    n_all = actp.tile([128, W], F32, tag="n_all")
    hmn = actp.tile([128, W], F32, tag="hmn")
    zh = actp.tile([128, W], F32, tag="zh")
    h1T_all = actp.tile([128, W], F32, tag="h1T_all")
    HW2 = W // 2
    for h in range(2):
        hs = slice(h * HW2, (h + 1) * HW2)
        gs = slice(2 * W + h * HW2, 2 * W + (h + 1) * HW2)
        nc.vector.tensor_add(gxfn[:, hs], gxa_all[:, gs], gx1[:, gs])
        nc.vector.tensor_mul(rhn[:, hs], r_all[:, hs], gh[:, gs])
        nc.vector.tensor_add(gxfn[:, hs], gxfn[:, hs], rhn[:, hs])
        nc.scalar.activation(n_all[:, hs], gxfn[:, hs], AF.Tanh)
        nc.vector.tensor_sub(hmn[:, hs], hTf_sb[:, hs], n_all[:, hs])
        nc.vector.tensor_mul(zh[:, hs], z_all[:, hs], hmn[:, hs])
        nc.vector.tensor_add(h1T_all[:, hs], n_all[:, hs], zh[:, hs])
        for ht in range(h * HT // 2, (h + 1) * HT // 2):
            ph = ms_ps.tile([BL, 128], F32, tag="ms")
            nc.tensor.transpose(
                ph[:], h1T_all[:, ht * BL : (ht + 1) * BL], identf[:]
            )
            nc.vector.tensor_copy(h1nat[:, 128 * ht : 128 * (ht + 1)], ph[:])
        nc.sync.dma_start(
            d["d_h1"][:, h * HID // 2 : (h + 1) * HID // 2],
            h1nat[:, h * HID // 2 : (h + 1) * HID // 2],
        )
    stack.close()


_NC_CACHE = None


def _get_program():
    global _NC_CACHE
    if _NC_CACHE is None:
        _NC_CACHE = build_program()
    return _NC_CACHE


def _ptile(x, np_dtype):
    """[T*128, rest...] -> [128, T*rest] with partition dim first."""
    x = np.asarray(x, np.float32)
    t = x.shape[0] // 128
    out = x.reshape(t, 128, -1).transpose(1, 0, 2).reshape(128, -1)
    return np.ascontiguousarray(out.astype(np_dtype))


def make_in_maps(inputs):
    """Host-side prep: shard batch across cores, transpose/fuse weights."""
    f = lambda x: np.ascontiguousarray(np.asarray(x, dtype=np.float32))
    bf = lambda x: np.ascontiguousarray(np.asarray(x, dtype=np.float32).astype(BF16NP))
    input_ = f(inputs["input"])
    hidden = f(inputs["hidden"])
    context = f(inputs["context"])
    mask = np.asarray(inputs["context_mask"])
    Wq, bq = f(inputs["Wq"]), f(inputs["bq"])
    Wk, bk = f(inputs["Wk"]), f(inputs["bk"])
    Wl = f(inputs["Wl"])
    We, be = f(inputs["We"]), f(inputs["be"])
    Wa, ba = f(inputs["Wa"]), f(inputs["ba"])
    W_ih, W_hh = f(inputs["W_ih"]), f(inputs["W_hh"])
    b_ih, b_hh = f(inputs["b_ih"]), f(inputs["b_hh"])

    wlrep = np.broadcast_to(
        Wl[0].reshape(AT, 128).T[:, :, None], (128, AT, 128)
    ).reshape(128, AT * 128)

    def wgroups_of(WT, np_dtype):
        # [1024, H] -> per group g: [:, 256g:256(g+1)] as [128, 8, 256]
        H = WT.shape[1]
        g = WT.reshape(8, 128, H).transpose(1, 0, 2).reshape(128, 8, H // 256, 256)
        return np.ascontiguousarray(
            g.transpose(2, 0, 1, 3).astype(np_dtype)  # [NG, 128, 8, 256]
        )

    wq_g = wgroups_of(Wq.T.astype(np.float32), BF16NP)
    w1_g = wgroups_of((W_ih @ We).T, BF16NP)
    whh_g = wgroups_of(W_hh.T, BF16NP)
    wgall = np.concatenate(
        [wq_g.reshape(4, 128, -1), w1_g.reshape(12, 128, -1),
         whh_g.reshape(12, 128, -1)], axis=0
    )
    w2_g = wgroups_of(((W_ih @ Wa).T * W2_SCALE).astype(np.float32), FP8NP)
    shared = {
        "wk8": _ptile(Wk.T * WK_SCALE, FP8NP).reshape(128, CT, ATT),
        "wgall": np.ascontiguousarray(wgall),
        "w2g": np.ascontiguousarray(w2_g),
        "wlrep": np.ascontiguousarray(wlrep.astype(BF16NP)),
        "bqk": _ptile((bq + bk).reshape(AT * 128, 1), np.float32),
        "bx": _ptile((W_ih @ (be + ba) + b_ih).reshape(H3, 1), np.float32),
        "bhh": _ptile(b_hh.reshape(H3, 1), np.float32),
        "identf": np.eye(128, dtype=np.float32),
        "ones1": np.ones((1, 128), BF16NP),
        "one1b": np.ones((1, 1), BF16NP),
    }
    pen = np.where(mask, np.float32(-1e18), np.float32(0.0)).astype(BF16NP)
    inT = np.ascontiguousarray(input_.T)
    hT = np.ascontiguousarray(hidden.T)

    in_maps = []
    for k in range(NCORES):
        sl = slice(k * BL, (k + 1) * BL)
        blk = context[sl]
        in_maps.append(
            {
                "ctxb": np.ascontiguousarray(
                    blk.reshape(BL, ST, 128, CTX).transpose(0, 2, 1, 3).astype(BF16NP)
                ),
                "ctxT8": np.ascontiguousarray(
                    blk.transpose(0, 2, 1).reshape(BL, CT, 128, S)
                    .transpose(0, 2, 1, 3).astype(FP8NP)
                ),
                "pen": np.ascontiguousarray(pen[sl].reshape(1, BL * S)),
                "inT": _ptile(inT[:, sl], BF16NP),
                "hT": _ptile(hT[:, sl], BF16NP),
                "hTf": _ptile(hT[:, sl], np.float32),
                **shared,
            }
        )
    return in_maps


def kernel(**inputs):
    nc = _get_program()
    in_maps = make_in_maps(inputs)
    res = run_bass_kernel_spmd(nc, in_maps, core_ids=list(range(NCORES)))
    hidden1 = np.concatenate([res.results[k]["h1"] for k in range(NCORES)], axis=0)
    attn = np.concatenate([res.results[k]["attn"] for k in range(NCORES)], axis=0)
    return (hidden1, attn)
